# revision 1
# baseline (speedup 1.0000x reference)
"""AttentionDecoder Trainium2 kernel: 8-way model-parallel LSTM+attention decoder.

Strategy:
  - Weights sharded 8 ways over the gate/hidden dims, SBUF-resident.
  - Activations feature-major [feat, batch]; matmuls are activation-stationary
    (lhsT = activation [K=feat, M=batch], rhs = weight.T [K=feat, N=out_feats]).
  - Per timestep: 3 AllGathers (h0, h1, o) across the 8 cores.
  - Attention refactored: M1T[b] = (h_enc[b] @ W1).T and M2[b] = h_enc[b] @ W2v.T
    are precomputed (hoists h_enc out of the sequential loop), so per step
    scores[b] = M1T[b].T @ h1[:,b] + c1[b] and
    z[b] = a[b] @ M2[b] + W2h @ h1[:,b] + b2,  o = tanh(z).
  - Per-core batch shard for attention: core k owns batch 8k..8k+7.
"""

import os
import warnings

warnings.filterwarnings("ignore")

import numpy as np

VOCAB, E, H, L, B, T, S, V = 32000, 512, 1024, 2, 64, 64, 128, 1024
NCORES = 8
P = 128
BG = B // NCORES  # 8 batch per core for attention
HC = H // NCORES  # 128 hidden feats per core
GC = 4 * HC  # 512 gate rows per core

REMOTE_MODE = int(os.environ.get("DEC_REMOTE", "0"))
USE_REMOTE = REMOTE_MODE >= 1

_COMPILED = None


def _build(n_steps: int):
    import concourse.bass as bass
    import concourse.bacc as bacc
    import concourse.mybir as mybir
    import concourse.tile as tile
    from concourse import masks

    fp32 = mybir.dt.float32
    AF = mybir.ActivationFunctionType
    AX = mybir.AxisListType

    nc = bacc.Bacc(
        "TRN2",
        target_bir_lowering=False,
        debug=False,
        num_devices=NCORES,
        monotonic_sem_count=12,
    )
    rsems = [nc.monotonic_semaphore(i).sem() for i in range(6)]  # h0e,h0o,h1e,h1o,oe,oo
    lsems = [nc.monotonic_semaphore(6 + i).sem() for i in range(6)]
    import os as _os
    _rm = int(_os.environ.get("DEC_REMOTE", "0"))
    RD = [(0, d) for d in range(NCORES)]
    if _rm == 2:
        RD = [None] + [(0, d) for d in range(1, NCORES)]
    RSEM_PER_ROUND = 14 if _rm == 2 else 16

    # ---- DRAM parameters (per-core data) ----
    d_xseq = nc.dram_tensor("xseq", [n_steps, P, 4, B], fp32, kind="ExternalInput")
    d_w0T = nc.dram_tensor("w0T", [16, P, GC], fp32, kind="ExternalInput")
    d_w1T = nc.dram_tensor("w1T", [16, P, GC], fp32, kind="ExternalInput")
    d_b0 = nc.dram_tensor("b0", [B, GC], fp32, kind="ExternalInput")
    d_b1 = nc.dram_tensor("b1", [B, GC], fp32, kind="ExternalInput")
    d_m1t = nc.dram_tensor("m1t", [BG, 8, P, S], fp32, kind="ExternalInput")
    d_c1t = nc.dram_tensor("c1t", [2, P, S], fp32, kind="ExternalInput")
    d_m2s = nc.dram_tensor("m2s", [BG, P, E], fp32, kind="ExternalInput")
    d_w2hT = nc.dram_tensor("w2hT", [8, P, E], fp32, kind="ExternalInput")
    d_b2 = nc.dram_tensor("b2", [BG, E], fp32, kind="ExternalInput")
    d_h0i = nc.dram_tensor("h0i", [P, 8, B], fp32, kind="ExternalInput")
    d_h1i = nc.dram_tensor("h1i", [P, 8, B], fp32, kind="ExternalInput")
    d_oi = nc.dram_tensor("oi", [P, NCORES * 4 * BG], fp32, kind="ExternalInput")
    d_out = nc.dram_tensor("out", [n_steps, BG, E], fp32, kind="ExternalOutput")

    with tile.TileContext(nc) as tc:
        import contextlib

        ctx = contextlib.ExitStack()
        with ctx:
            wpool = ctx.enter_context(tc.tile_pool(name="weights", bufs=1))
            spool = ctx.enter_context(tc.tile_pool(name="state", bufs=1))
            xpool = ctx.enter_context(tc.tile_pool(name="x", bufs=2))
            tpool = ctx.enter_context(tc.tile_pool(name="tmp", bufs=2))
            ppool = ctx.enter_context(tc.tile_pool(name="psum", bufs=2, space="PSUM"))
            p1pool = ctx.enter_context(tc.tile_pool(name="psum1", bufs=1, space="PSUM"))
            dpool = ctx.enter_context(tc.tile_pool(name="dram", bufs=2, space="DRAM"))

            # ---- persistent SBUF tiles ----
            w0T = wpool.tile([P, 16, GC], fp32, tag="w0T")
            w1T = wpool.tile([P, 16, GC], fp32, tag="w1T")
            b0 = wpool.tile([B, GC], fp32, tag="b0")
            b1 = wpool.tile([B, GC], fp32, tag="b1")
            m1t = wpool.tile([P, BG, 8, S], fp32, tag="m1t")
            c1t = wpool.tile([P, 2, S], fp32, tag="c1t")
            m2s = wpool.tile([P, BG, E], fp32, tag="m2s")
            w2hT = wpool.tile([P, 8, E], fp32, tag="w2hT")
            b2 = wpool.tile([BG, E], fp32, tag="b2")
            ident = wpool.tile([P, P], fp32, tag="ident")

            h0f = [spool.tile([P, NCORES * B], fp32, tag=f"h0f{i}", name=f"h0f{i}") for i in range(2)]
            h1f = [spool.tile([P, NCORES * B], fp32, tag=f"h1f{i}", name=f"h1f{i}") for i in range(2)]
            of = [spool.tile([P, NCORES * 4 * BG], fp32, tag=f"of{i}", name=f"of{i}") for i in range(2)]
            c0 = spool.tile([B, HC], fp32, tag="c0")  # cell states, batch-major
            c1 = spool.tile([B, HC], fp32, tag="c1")
            h1my = spool.tile([P, 8, BG], fp32, tag="h1my")

            # ---- prologue loads ----
            nc.sync.dma_start(w0T[:], d_w0T[:].rearrange("kt p g -> p kt g"))
            nc.sync.dma_start(w1T[:], d_w1T[:].rearrange("kt p g -> p kt g"))
            nc.sync.dma_start(b0[:], d_b0[:])
            nc.sync.dma_start(b1[:], d_b1[:])
            nc.scalar.dma_start(m1t[:], d_m1t[:].rearrange("j kt p s -> p j kt s"))
            nc.scalar.dma_start(m2s[:], d_m2s[:].rearrange("j p e -> p j e"))
            nc.sync.dma_start(c1t[:], d_c1t[:].rearrange("h p s -> p h s"))
            nc.scalar.dma_start(w2hT[:], d_w2hT[:].rearrange("kt p e -> p kt e"))
            nc.sync.dma_start(b2[:], d_b2[:])
            masks.make_identity(nc, ident[:])
            nc.sync.dma_start(h0f[1][:].rearrange("p (kc b) -> p kc b", kc=8), d_h0i[:])
            nc.sync.dma_start(h1f[1][:].rearrange("p (kc b) -> p kc b", kc=8), d_h1i[:])
            # of init: slot k holds o columns [c, j] for b-group k
            nc.sync.dma_start(of[1][:], d_oi[:])
            nc.vector.memset(c0[:], 0.0)
            nc.vector.memset(c1[:], 0.0)

            pid = nc.vector.partition_id()
            pid_pl = nc.gpsimd.partition_id()

            def lstm_pointwise(g_sb, cst, h_out):
                """g_sb [B, 4*HC] gate order i,f,g,o; updates cst, writes h_out [B,HC]."""
                gt = tpool.tile([B, HC], fp32, tag="pw_gt")
                ot = tpool.tile([B, HC], fp32, tag="pw_ot")
                ift = tpool.tile([B, 2 * HC], fp32, tag="pw_ift")
                nc.scalar.activation(ift[:], g_sb[:, 0 : 2 * HC], AF.Sigmoid)
                it, ft = ift[:, 0:HC], ift[:, HC : 2 * HC]
                nc.scalar.activation(gt[:], g_sb[:, 2 * HC : 3 * HC], AF.Tanh)
                nc.scalar.activation(ot[:], g_sb[:, 3 * HC : 4 * HC], AF.Sigmoid)
                t1 = tpool.tile([B, HC], fp32, tag="pw_t1")
                nc.vector.tensor_mul(t1[:], ft, cst[:])
                nc.vector.tensor_mul(gt[:], it, gt[:])
                nc.vector.tensor_add(cst[:], t1[:], gt[:])
                tc_ = tpool.tile([B, HC], fp32, tag="pw_tc")
                nc.scalar.activation(tc_[:], cst[:], AF.Tanh)
                nc.vector.tensor_mul(h_out[:], ot[:], tc_[:])

            def evict_src(t, kind, dst_ap, src_ap):
                if t >= 2 and USE_REMOTE:
                    with tc.tile_critical():
                        nc.vector.wait_ge(lsems[2 * kind + (t % 2)], 16 * (t // 2))
                        nc.vector.tensor_copy(dst_ap, src_ap)
                else:
                    nc.vector.tensor_copy(dst_ap, src_ap)

            def exchange(t, kind, src_sb, width, dst_tile):
                """Broadcast my [P,width] chunk into slot pid of everyone's dst_tile."""
                if not USE_REMOTE:
                    bi = dpool.tile([P, width], fp32, tag=f"agi{kind}", name=f"agi{kind}")
                    bo = dpool.tile(
                        [P * NCORES, width], fp32, tag=f"ago{kind}", name=f"ago{kind}"
                    )
                    nc.gpsimd.dma_start(bi[:], src_sb)
                    nc.gpsimd.collective_compute(
                        "AllGather",
                        mybir.AluOpType.bypass,
                        replica_groups=[list(range(NCORES))],
                        ins=[bi.opt()],
                        outs=[bo.opt()],
                    )
                    nc.gpsimd.dma_start(
                        dst_tile[:].rearrange("p (k w) -> p k w", k=NCORES),
                        bo[:].rearrange("(k p) w -> p k w", p=P),
                    )
                    return
                rsem = rsems[2 * kind + (t % 2)]
                nc.gpsimd.remote_dma_broadcast(
                    dst_tile[:, bass.ts(pid_pl, width)],
                    src_sb,
                    rsem,
                    lsems[2 * kind + (t % 2)],
                    rdests=RD,
                )
                nc.gpsimd.trigger_dma(count=None)
                if RSEM_PER_ROUND == 14:
                    # self slot not broadcast; copy locally
                    nc.vector.tensor_copy(
                        dst_tile[:, bass.ts(pid, width)], src_sb
                    )
                with tc.tile_critical():
                    nc.vector.wait_ge(rsem, RSEM_PER_ROUND * (t // 2 + 1))
                    nc.vector.tensor_copy(dst_tile[0:1, 0:1], dst_tile[0:1, 0:1])

            for t in range(n_steps):
                # ---- x load ----
                xt = xpool.tile([P, 4, B], fp32, tag="xt")
                nc.scalar.dma_start(xt[:], d_xseq[t])

                # ---- gates0: K = [x(4) | o(4) | h0(8)] ----
                h0f_r = h0f[(t - 1) % 2]
                h1f_r = h1f[(t - 1) % 2]
                of_r = of[(t - 1) % 2]
                of_rv = of_r[:].rearrange("p (k c j) -> p c k j", k=NCORES, c=4)
                o4 = tpool.tile([P, 4, B], fp32, tag="o4")
                nc.vector.tensor_copy(
                    o4[:].rearrange("p c (k j) -> p c k j", k=NCORES), of_rv
                )
                pg0 = ppool.tile([P, 2, GC], fp32, tag="pg")
                order0 = [0, 1, 2, 3] + [8, 9, 10, 11, 12, 13, 14, 15] + [4, 5, 6, 7]
                for i, kt in enumerate(order0):
                    if kt < 4:
                        lhsT = xt[:, kt, :]
                    elif kt < 8:
                        lhsT = o4[:, kt - 4, :]
                    else:
                        lhsT = h0f_r[:, (kt - 8) * B : (kt - 7) * B]
                    hf = i % 2
                    nc.tensor.matmul(
                        pg0[64 * hf : 64 * hf + 64, hf, :],
                        lhsT,
                        w0T[:, kt, :],
                        start=(i < 2),
                        stop=(i >= 14),
                        tile_position=(0, 64 * hf),
                    )
                g0 = tpool.tile([B, GC], fp32, tag="g0")
                nc.vector.tensor_add(g0[:], pg0[0:64, 0, :], b0[:])
                nc.vector.tensor_add(g0[:], g0[:], pg0[64:128, 1, :])
                h0m = tpool.tile([B, HC], fp32, tag="h0m")
                lstm_pointwise(g0, c0, h0m)

                # ---- transpose h0m -> [HC, B], AG -> h0f ----
                pt0 = p1pool.tile([P, 128], fp32, tag="ptr", name="pt0")
                nc.tensor.transpose(pt0[:, 0:B], h0m[:], ident[0:B, 0:B])
                h0T = tpool.tile([P, B], fp32, tag="h0T")
                evict_src(t, 0, h0T[:], pt0[:, 0:B])
                h0src = h0T[:]

                exchange(t, 0, h0src, B, h0f[t % 2])

                # ---- gates1: K = [h0(8) | h1(8)] ----
                h0f_w = h0f[t % 2]
                pg1 = ppool.tile([P, 2, GC], fp32, tag="pg")
                order1 = [8, 9, 10, 11, 12, 13, 14, 15] + [0, 1, 2, 3, 4, 5, 6, 7]
                for i, kt in enumerate(order1):
                    lhsT = (
                        h0f_w[:, kt * B : (kt + 1) * B]
                        if kt < 8
                        else h1f_r[:, (kt - 8) * B : (kt - 7) * B]
                    )
                    hf = i % 2
                    nc.tensor.matmul(
                        pg1[64 * hf : 64 * hf + 64, hf, :],
                        lhsT,
                        w1T[:, kt, :],
                        start=(i < 2),
                        stop=(i >= 14),
                        tile_position=(0, 64 * hf),
                    )
                g1 = tpool.tile([B, GC], fp32, tag="g1")
                nc.vector.tensor_add(g1[:], pg1[0:64, 0, :], b1[:])
                nc.vector.tensor_add(g1[:], g1[:], pg1[64:128, 1, :])
                h1m = tpool.tile([B, HC], fp32, tag="h1m")
                lstm_pointwise(g1, c1, h1m)

                # ---- transpose h1m, AG -> h1f ----
                pt1 = p1pool.tile([P, 128], fp32, tag="ptr", name="pt1")
                nc.tensor.transpose(pt1[:, 0:B], h1m[:], ident[0:B, 0:B])
                h1T = tpool.tile([P, B], fp32, tag="h1T")
                evict_src(t, 1, h1T[:], pt1[:, 0:B])
                h1src = h1T[:]

                exchange(t, 1, h1src, B, h1f[t % 2])

                # ---- select my batch columns of h1 (query) ----
                h1f_wv = h1f[t % 2][:].rearrange("p (kc b) -> p kc b", kc=8)
                nc.vector.tensor_copy(h1my[:], h1f_wv[:, :, bass.ts(pid, BG)])

                # ---- scores: per-b matvec via tile_position packing ----
                psc = p1pool.tile([P, 2, S], fp32, tag="psc")
                nc.vector.memset(psc[:], 0.0)
                for j in range(BG):
                    half, row = j // 4, 32 * (j % 4)
                    for kt in range(8):
                        nc.tensor.matmul(
                            psc[row : row + 1, half, :],
                            h1my[:, kt, j : j + 1],
                            m1t[:, j, kt, :],
                            start=(kt == 0),
                            stop=(kt == 7),
                            tile_position=(0, row),
                        )
                # ---- softmax over the two halves (garbage rows are fine) ----
                a_sb = tpool.tile([P, 2, S], fp32, tag="a_sb")
                stat = tpool.tile([P, 4], fp32, tag="stat")
                for half in range(2):
                    nc.vector.tensor_add(
                        a_sb[:, half, :], psc[:, half, :], c1t[:, half, :]
                    )
                    nm = stat[:, 2 * half : 2 * half + 1]
                    nc.vector.tensor_reduce(
                        nm, a_sb[:, half, :], axis=AX.X, op=mybir.AluOpType.max,
                        negate=True,
                    )
                    sm = stat[:, 2 * half + 1 : 2 * half + 2]
                    nc.scalar.activation(
                        a_sb[:, half, :], a_sb[:, half, :], AF.Exp, bias=nm,
                        accum_out=sm,
                    )
                    nc.vector.reciprocal(sm, sm)
                    nc.vector.tensor_scalar_mul(a_sb[:, half, :], a_sb[:, half, :], sm)

                # ---- transpose a -> columns; build block-diag lhsT ----
                paT = p1pool.tile([P, 2, S], fp32, tag="psc", name="paT")
                nc.tensor.transpose(paT[:, 0, :], a_sb[:, 0, :], ident[:])
                nc.tensor.transpose(paT[:, 1, :], a_sb[:, 1, :], ident[:])
                abd = tpool.tile([P, BG * BG], fp32, tag="abd")
                nc.vector.memset(abd[:], 0.0)
                # dst cols 9j <- paT cols 128*(j//4) + 32*(j%4), one strided copy
                nc.vector.tensor_copy(
                    abd[:, 0 : BG * BG : 9].rearrange("p (a b) -> p a b", a=2),
                    paT[:].rearrange("p h (c x) -> p h c x", c=4)[:, :, :, 0:1],
                )

                # ---- z = blockdiag(a) @ M2stack + h1my.T @ W2h.T ----
                pz = p1pool.tile([BG, E], fp32, tag="pz")
                for j in range(BG):
                    nc.tensor.matmul(
                        pz[:],
                        abd[:, j * BG : (j + 1) * BG],
                        m2s[:, j, :],
                        start=(j == 0),
                        stop=False,
                    )
                for kt in range(8):
                    nc.tensor.matmul(
                        pz[:], h1my[:, kt, :], w2hT[:, kt, :], start=False,
                        stop=(kt == 7),
                    )
                zt = tpool.tile([BG, E], fp32, tag="zt")
                nc.vector.tensor_add(zt[:], pz[:], b2[:])
                o_sb = tpool.tile([BG, E], fp32, tag="o_sb")
                nc.scalar.activation(o_sb[:], zt[:], AF.Tanh)

                # ---- write output ----
                nc.scalar.dma_start(d_out[t], o_sb[:])

                # ---- transpose o chunks -> [P, 4, BG], AG -> of ----
                poT = p1pool.tile([P, 4, BG], fp32, tag="ptr", name="poT")
                for cchunk in range(4):
                    nc.tensor.transpose(
                        poT[:, cchunk, :],
                        o_sb[:, cchunk * P : (cchunk + 1) * P],
                        ident[0:BG, 0:BG],
                    )
                oT = tpool.tile([P, 4 * BG], fp32, tag="oT")
                evict_src(t, 2, oT[:].rearrange("p (c j) -> p c j", c=4), poT[:])
                osrc = oT[:]

                exchange(t, 2, osrc, 4 * BG, of[t % 2])

    nc.compile()
    return nc


def _host_prep(inputs: dict, n_steps: int):
    """Build per-core in_maps."""
    f32 = np.float32
    tgt = np.asarray(inputs["tgt_batch"])
    h_enc = np.asarray(inputs["h_encoder"], f32)
    emb = np.asarray(inputs["emb"], f32)
    out_init = np.asarray(inputs["output_init"], f32)
    hid_init = np.asarray(inputs["hidden_init"], f32)
    W_ih = np.asarray(inputs["W_ih"], f32)
    W_hh = np.asarray(inputs["W_hh"], f32)
    b_ih = np.asarray(inputs["b_ih"], f32)
    b_hh = np.asarray(inputs["b_hh"], f32)
    W1 = np.asarray(inputs["W1"], f32)
    b1v = np.asarray(inputs["b1"], f32)
    W2 = np.asarray(inputs["W2"], f32)
    b2v = np.asarray(inputs["b2"], f32)

    # x sequence, feature-major, folded [T, P, 4, B]
    xs = emb[tgt[:n_steps]]  # [T, B, E]
    xseq = np.ascontiguousarray(
        xs.transpose(0, 2, 1).reshape(n_steps, 4, P, B).transpose(0, 2, 1, 3)
    )

    # full o / h inits, feature-major folds
    # oi[p, (k, c, j)] = o[c*128+p, 8k+j]
    oi4 = out_init.T.reshape(4, P, NCORES, 8)  # [c, p, k, j]
    oi = np.ascontiguousarray(oi4.transpose(1, 2, 0, 3).reshape(P, NCORES * 4 * 8))
    h0i = np.ascontiguousarray(hid_init[0].T.reshape(8, P, B).transpose(1, 0, 2))
    h1i = np.ascontiguousarray(hid_init[1].T.reshape(8, P, B).transpose(1, 0, 2))

    # attention precompute (host for now; small fraction of FLOPs)
    # M1T[b] = (h_enc[b] @ W1).T  [H, S];  c1[b] = h_enc[b] @ b1  [S]
    # M2[b] = h_enc[b] @ W2v.T  [S, E]
    W2v, W2h = W2[:, :V], W2[:, V:]
    M1T = np.einsum("bsv,vh->bhs", h_enc, W1).astype(f32)  # [B, H, S]
    c1v = np.einsum("bsv,v->bs", h_enc, b1v).astype(f32)  # [B, S]
    M2 = np.einsum("bsv,ev->bse", h_enc, W2v).astype(f32)  # [B, S, E]

    in_maps = []
    for k in range(NCORES):
        rows = np.concatenate([np.arange(g * H + k * HC, g * H + (k + 1) * HC) for g in range(4)])
        W0c = np.concatenate([W_ih[0], W_hh[0]], axis=1)[rows]  # [GC, 2048]
        W1c = np.concatenate([W_ih[1], W_hh[1]], axis=1)[rows]
        w0T = np.ascontiguousarray(W0c.T.reshape(16, P, GC))
        w1T = np.ascontiguousarray(W1c.T.reshape(16, P, GC))
        b0c = np.broadcast_to((b_ih[0] + b_hh[0])[rows], (B, GC)).copy()
        b1c = np.broadcast_to((b_ih[1] + b_hh[1])[rows], (B, GC)).copy()

        bs = np.arange(k * BG, (k + 1) * BG)
        m1tc = np.ascontiguousarray(M1T[bs].reshape(BG, 8, P, S))
        m2sc = np.ascontiguousarray(M2[bs])  # [BG, S, E] (S=P)
        c1tc = np.zeros((2, P, S), f32)
        for j in range(BG):
            c1tc[j // 4, 32 * (j % 4), :] = c1v[bs[j]]
        w2hT = np.ascontiguousarray(W2h.T.reshape(8, P, E))
        b2c = np.broadcast_to(b2v, (BG, E)).copy()

        in_maps.append(
            {
                "xseq": xseq,
                "w0T": w0T,
                "w1T": w1T,
                "b0": b0c,
                "b1": b1c,
                "m1t": m1tc,
                "c1t": c1tc,
                "m2s": m2sc,
                "w2hT": w2hT,
                "b2": b2c,
                "h0i": h0i,
                "h1i": h1i,
                "oi": oi,
            }
        )
    return in_maps


def run(inputs: dict, n_steps: int = T, trace: bool = False):
    global _COMPILED
    from concourse.bass_utils import run_bass_kernel_spmd

    if _COMPILED is None or _COMPILED[1] != n_steps:
        _COMPILED = (_build(n_steps), n_steps)
    nc = _COMPILED[0]
    in_maps = _host_prep(inputs, n_steps)
    res = run_bass_kernel_spmd(
        nc, in_maps, core_ids=list(range(NCORES)), trace=trace
    )
    outs = [res.results[k]["out"] for k in range(NCORES)]  # [T, BG, E] each
    full = np.concatenate(outs, axis=1)  # [T, B, E]
    return np.ascontiguousarray(full.transpose(1, 0, 2)), res  # [B, T, E]


def kernel(**inputs) -> np.ndarray:
    out, _ = run(inputs, T)
    return out.astype(np.float32)



# revision 9
# speedup vs baseline: 3.5531x; 3.5531x over previous
"""AttentionDecoder Trainium2 kernel: 8-way model-parallel LSTM+attention decoder.

Strategy:
  - Weights sharded 8 ways over the gate/hidden dims, SBUF-resident.
  - Activations feature-major [feat, batch]; matmuls are activation-stationary
    (lhsT = activation [K=feat, M=batch], rhs = weight.T [K=feat, N=out_feats]).
  - Per timestep: 3 AllGathers (h0, h1, o) across the 8 cores.
  - Attention refactored: M1T[b] = (h_enc[b] @ W1).T and M2[b] = h_enc[b] @ W2v.T
    are precomputed ON DEVICE at the prologue (hoists h_enc out of the
    sequential loop), so per step
    scores[b] = M1T[b].T @ h1[:,b] + c1[b] and
    z[b] = a[b] @ M2[b] + W2h @ h1[:,b] + b2,  o = tanh(z).
  - Per-core batch shard for attention: core k owns batch 8k..8k+7.
  - Wire format is fp16 (the host->device tunnel is the bottleneck):
    LSTM weights upcast to fp32 on device (step-loop math unchanged),
    attention operands stay fp16 (psum accumulation is fp32).
  - Replicated arrays (xseq, W1, W2T) are sharded on the wire and
    AllGathered once on device at the prologue.
"""

import os
import warnings

warnings.filterwarnings("ignore")

import numpy as np

VOCAB, E, H, L, B, T, S, V = 32000, 512, 1024, 2, 64, 64, 128, 1024
NCORES = 8
P = 128
BG = B // NCORES  # 8 batch per core for attention
HC = H // NCORES  # 128 hidden feats per core
GC = 4 * HC  # 512 gate rows per core
W2R = (H + V) // NCORES  # 256 rows of W2T per core

REMOTE_MODE = int(os.environ.get("DEC_REMOTE", "0"))
USE_REMOTE = REMOTE_MODE >= 1

_COMPILED = None


def _build(n_steps: int):
    import concourse.bass as bass
    import concourse.bacc as bacc
    import concourse.mybir as mybir
    import concourse.tile as tile
    from concourse import masks

    fp32 = mybir.dt.float32
    fp16 = mybir.dt.float16
    AF = mybir.ActivationFunctionType
    AX = mybir.AxisListType

    npc = max(1, (n_steps + NCORES - 1) // NCORES)  # steps per core on the wire
    n_pad = npc * NCORES

    nc = bacc.Bacc(
        "TRN2",
        target_bir_lowering=False,
        debug=False,
        num_devices=NCORES,
        monotonic_sem_count=12,
    )
    rsems = [nc.monotonic_semaphore(i).sem() for i in range(6)]  # h0e,h0o,h1e,h1o,oe,oo
    lsems = [nc.monotonic_semaphore(6 + i).sem() for i in range(6)]
    import os as _os
    _rm = int(_os.environ.get("DEC_REMOTE", "0"))
    RD = [(0, d) for d in range(NCORES)]
    if _rm == 2:
        RD = [None] + [(0, d) for d in range(1, NCORES)]
    RSEM_PER_ROUND = 14 if _rm == 2 else 16

    # ---- DRAM parameters (per-core data, fp16 wire format) ----
    d_xs8 = nc.dram_tensor("xs8", [npc, P, 4, B], fp16, kind="ExternalInput")
    d_w0T = nc.dram_tensor("w0T", [16, P, GC], fp16, kind="ExternalInput")
    d_w1T = nc.dram_tensor("w1T", [16, P, GC], fp16, kind="ExternalInput")
    d_henc = nc.dram_tensor("henc", [BG, S, V], fp16, kind="ExternalInput")
    d_w1s = nc.dram_tensor("w1s", [P, H], fp16, kind="ExternalInput")
    d_w2Ts = nc.dram_tensor("w2Ts", [W2R, E], fp16, kind="ExternalInput")
    d_bias = nc.dram_tensor("biasv", [1, 2 * GC + E + V], fp32, kind="ExternalInput")
    d_h0i = nc.dram_tensor("h0i", [P, 8, B], fp16, kind="ExternalInput")
    d_h1i = nc.dram_tensor("h1i", [P, 8, B], fp16, kind="ExternalInput")
    d_oi = nc.dram_tensor("oi", [P, NCORES * 4 * BG], fp16, kind="ExternalInput")
    d_out = nc.dram_tensor("out", [n_steps, BG, E], fp16, kind="ExternalOutput")

    with tile.TileContext(nc) as tc:
        import contextlib

        ctx = contextlib.ExitStack()
        with ctx:
            wpool = ctx.enter_context(tc.tile_pool(name="weights", bufs=1))
            spool = ctx.enter_context(tc.tile_pool(name="state", bufs=1))
            xpool = ctx.enter_context(tc.tile_pool(name="x", bufs=2))
            tpool = ctx.enter_context(tc.tile_pool(name="tmp", bufs=2))
            ppool = ctx.enter_context(tc.tile_pool(name="psum", bufs=2, space="PSUM"))
            p1pool = ctx.enter_context(tc.tile_pool(name="psum1", bufs=1, space="PSUM"))
            dpool = ctx.enter_context(tc.tile_pool(name="dram", bufs=2, space="DRAM"))

            # ---- persistent SBUF tiles ----
            w0T = wpool.tile([P, 16, GC], fp32, tag="w0T")
            w1T = wpool.tile([P, 16, GC], fp32, tag="w1T")
            b0 = wpool.tile([B, GC], fp32, tag="b0")
            b1 = wpool.tile([B, GC], fp32, tag="b1")
            m1t = wpool.tile([P, BG, 8, S], fp16, tag="m1t")
            c1t = wpool.tile([P, 2, S], fp32, tag="c1t")
            m2s = wpool.tile([P, BG, E], fp16, tag="m2s")
            w2hT = wpool.tile([P, 8, E], fp16, tag="w2hT")
            b2 = wpool.tile([BG, E], fp32, tag="b2")
            ident = wpool.tile([P, P], fp32, tag="ident")
            id16 = wpool.tile([P, P], fp16, tag="id16")
            b1colf = wpool.tile([P, 8], fp32, tag="b1colf")
            b1col = wpool.tile([P, 8], fp16, tag="b1col")

            h0f = [spool.tile([P, NCORES * B], fp32, tag=f"h0f{i}", name=f"h0f{i}") for i in range(2)]
            h1f = [spool.tile([P, NCORES * B], fp32, tag=f"h1f{i}", name=f"h1f{i}") for i in range(2)]
            of = [spool.tile([P, NCORES * 4 * BG], fp32, tag=f"of{i}", name=f"of{i}") for i in range(2)]
            c0 = spool.tile([B, HC], fp32, tag="c0")  # cell states, batch-major
            c1 = spool.tile([B, HC], fp32, tag="c1")
            h1my = spool.tile([P, 8, BG], fp16, tag="h1my")

            # ---- prologue: weights (fp16 wire -> fp32 SBUF) ----
            wstage = wpool.tile([P, 16, GC], fp16, tag="wstage", name="wstage0")
            nc.sync.dma_start(wstage[:], d_w0T[:].rearrange("kt p g -> p kt g"))
            nc.vector.tensor_copy(w0T[:], wstage[:])
            wstage2 = wpool.tile([P, 16, GC], fp16, tag="wstage", name="wstage1")
            nc.sync.dma_start(wstage2[:], d_w1T[:].rearrange("kt p g -> p kt g"))
            nc.vector.tensor_copy(w1T[:], wstage2[:])

            masks.make_identity(nc, ident[:])
            nc.vector.tensor_copy(id16[:], ident[:])

            # ---- prologue: biases via K=1 outer-product broadcast ----
            bstage = wpool.tile([1, 2 * GC + E + V], fp32, tag="bstage")
            nc.sync.dma_start(bstage[:], d_bias[:])
            onesB = wpool.tile([1, B], fp32, tag="onesB")
            nc.vector.memset(onesB[:], 1.0)
            pb = ppool.tile([P, 2, GC], fp32, tag="pg", name="pb")
            nc.tensor.matmul(pb[0:B, 0, :], onesB[0:1, :], bstage[0:1, 0:GC],
                             start=True, stop=True)
            nc.vector.tensor_copy(b0[:], pb[0:B, 0, :])
            pb2 = ppool.tile([P, 2, GC], fp32, tag="pg", name="pb2")
            nc.tensor.matmul(pb2[0:B, 0, :], onesB[0:1, :], bstage[0:1, GC:2 * GC],
                             start=True, stop=True)
            nc.vector.tensor_copy(b1[:], pb2[0:B, 0, :])
            pb3 = ppool.tile([P, 2, GC], fp32, tag="pg", name="pb3")
            nc.tensor.matmul(pb3[0:BG, 0, :], onesB[0:1, 0:BG],
                             bstage[0:1, 2 * GC:2 * GC + E], start=True, stop=True)
            nc.vector.tensor_copy(b2[:], pb3[0:BG, 0, :])
            # b1 attention vector -> [P, vc] column layout, cast to fp16
            nc.sync.dma_start(
                b1colf[:], d_bias[0, 2 * GC + E:].rearrange("(vc p) -> p vc", p=P)
            )
            nc.vector.tensor_copy(b1col[:], b1colf[:])

            # ---- prologue: AllGather sharded uploads (xseq, W1, W2T) ----
            # (collectives cannot read IO tensors directly; stage via
            #  internal DRAM tiles first)
            sxs = dpool.tile([npc, P, 4, B], fp16, tag="sxs")
            nc.sync.dma_start(sxs[:], d_xs8[:])
            agx = dpool.tile([n_pad, P, 4, B], fp16, tag="agx")
            nc.gpsimd.collective_compute(
                "AllGather",
                mybir.AluOpType.bypass,
                replica_groups=[list(range(NCORES))],
                ins=[sxs.opt()],
                outs=[agx.opt()],
            )
            sw1 = dpool.tile([P, H], fp16, tag="sw1")
            nc.sync.dma_start(sw1[:], d_w1s[:])
            agw1 = dpool.tile([H, H], fp16, tag="agw1")
            nc.gpsimd.collective_compute(
                "AllGather",
                mybir.AluOpType.bypass,
                replica_groups=[list(range(NCORES))],
                ins=[sw1.opt()],
                outs=[agw1.opt()],
            )
            sw2 = dpool.tile([W2R, E], fp16, tag="sw2")
            nc.sync.dma_start(sw2[:], d_w2Ts[:])
            agw2 = dpool.tile([H + V, E], fp16, tag="agw2")
            nc.gpsimd.collective_compute(
                "AllGather",
                mybir.AluOpType.bypass,
                replica_groups=[list(range(NCORES))],
                ins=[sw2.opt()],
                outs=[agw2.opt()],
            )

            w1sb = wpool.tile([P, 8, H], fp16, tag="wstage", name="w1sb")
            nc.scalar.dma_start(w1sb[:], agw1[:].rearrange("(vc p) h -> p vc h", p=P))
            w2vT = wpool.tile([P, 8, E], fp16, tag="w2vT")
            nc.scalar.dma_start(
                w2vT[:], agw2[0:V, :].rearrange("(vc p) e -> p vc e", p=P)
            )
            nc.scalar.dma_start(
                w2hT[:], agw2[V:, :].rearrange("(hc p) e -> p hc e", p=P)
            )

            # ---- prologue: init states (fp16 wire -> fp32 SBUF) ----
            hstage = wpool.tile([P, 8, B], fp16, tag="hstage")
            nc.sync.dma_start(hstage[:], d_h0i[:])
            nc.vector.tensor_copy(
                h0f[1][:].rearrange("p (kc b) -> p kc b", kc=8), hstage[:]
            )
            hstage2 = wpool.tile([P, 8, B], fp16, tag="hstage2")
            nc.sync.dma_start(hstage2[:], d_h1i[:])
            nc.vector.tensor_copy(
                h1f[1][:].rearrange("p (kc b) -> p kc b", kc=8), hstage2[:]
            )
            ostage = wpool.tile([P, NCORES * 4 * BG], fp16, tag="ostage")
            nc.sync.dma_start(ostage[:], d_oi[:])
            nc.vector.tensor_copy(of[1][:], ostage[:])
            nc.vector.memset(c0[:], 0.0)
            nc.vector.memset(c1[:], 0.0)

            # ---- prologue: attention precompute on device ----
            # per owned batch b: m1t[b] = W1.T @ h_enc[b].T   [H, S]
            #                    m2s[b] = h_enc[b] @ W2v.T    [S, E]
            #                    c1[b]  = h_enc[b] @ b1       [S]
            nc.vector.memset(c1t[:], 0.0)
            for b in range(BG):
                hencb = tpool.tile([P, V], fp16, tag="hencb")
                nc.scalar.dma_start(hencb[:], d_henc[b])
                hencT = tpool.tile([P, 8, S], fp16, tag="hencT")
                for vc in range(8):
                    ptp = p1pool.tile([P, S], fp16, tag="ptr", name=f"ptp{b}_{vc}")
                    nc.tensor.transpose(
                        ptp[:], hencb[:, vc * P:(vc + 1) * P], id16[:]
                    )
                    nc.vector.tensor_copy(hencT[:, vc, :], ptp[:])
                # m1t: 8 h-chunks, accumulate over 8 v-chunks
                for hc in range(8):
                    pm = p1pool.tile([P, S], fp32, tag="ptr", name=f"pm{b}_{hc}")
                    for vc in range(8):
                        nc.tensor.matmul(
                            pm[:],
                            w1sb[:, vc, hc * P:(hc + 1) * P],
                            hencT[:, vc, :],
                            start=(vc == 0),
                            stop=(vc == 7),
                        )
                    nc.vector.tensor_copy(m1t[:, b, hc, :], pm[:])
                # m2s: accumulate over 8 v-chunks
                pm2 = p1pool.tile([P, E], fp32, tag="pz", name=f"pm2{b}")
                for vc in range(8):
                    nc.tensor.matmul(
                        pm2[:],
                        hencT[:, vc, :],
                        w2vT[:, vc, :],
                        start=(vc == 0),
                        stop=(vc == 7),
                    )
                nc.vector.tensor_copy(m2s[:, b, :], pm2[:])
                # c1 row: single-row matvec into partition 32*(b%4), half b//4
                half, row = b // 4, 32 * (b % 4)
                pc1 = p1pool.tile([P, 2, S], fp32, tag="psc", name=f"pc1{b}")
                for vc in range(8):
                    nc.tensor.matmul(
                        pc1[row:row + 1, half, :],
                        b1col[:, vc:vc + 1],
                        hencT[:, vc, :],
                        start=(vc == 0),
                        stop=(vc == 7),
                        tile_position=(0, row),
                    )
                nc.vector.tensor_copy(
                    c1t[row:row + 1, half, :], pc1[row:row + 1, half, :]
                )

            pid = nc.vector.partition_id()
            pid_pl = nc.gpsimd.partition_id()

            def lstm_pointwise(g_sb, cst, h_out):
                """g_sb [B, 4*HC] gate order i,f,g,o; updates cst, writes h_out [B,HC]."""
                gt = tpool.tile([B, HC], fp32, tag="pw_gt")
                ot = tpool.tile([B, HC], fp32, tag="pw_ot")
                ift = tpool.tile([B, 2 * HC], fp32, tag="pw_ift")
                nc.scalar.activation(ift[:], g_sb[:, 0 : 2 * HC], AF.Sigmoid)
                it, ft = ift[:, 0:HC], ift[:, HC : 2 * HC]
                nc.scalar.activation(gt[:], g_sb[:, 2 * HC : 3 * HC], AF.Tanh)
                nc.scalar.activation(ot[:], g_sb[:, 3 * HC : 4 * HC], AF.Sigmoid)
                t1 = tpool.tile([B, HC], fp32, tag="pw_t1")
                nc.vector.tensor_mul(t1[:], ft, cst[:])
                nc.vector.tensor_mul(gt[:], it, gt[:])
                nc.vector.tensor_add(cst[:], t1[:], gt[:])
                tc_ = tpool.tile([B, HC], fp32, tag="pw_tc")
                nc.scalar.activation(tc_[:], cst[:], AF.Tanh)
                nc.vector.tensor_mul(h_out[:], ot[:], tc_[:])

            def evict_src(t, kind, dst_ap, src_ap):
                if t >= 2 and USE_REMOTE:
                    with tc.tile_critical():
                        nc.vector.wait_ge(lsems[2 * kind + (t % 2)], 16 * (t // 2))
                        nc.vector.tensor_copy(dst_ap, src_ap)
                else:
                    nc.vector.tensor_copy(dst_ap, src_ap)

            def exchange(t, kind, src_sb, width, dst_tile):
                """Broadcast my [P,width] chunk into slot pid of everyone's dst_tile."""
                if not USE_REMOTE:
                    bi = dpool.tile([P, width], fp32, tag=f"agi{kind}", name=f"agi{kind}")
                    bo = dpool.tile(
                        [P * NCORES, width], fp32, tag=f"ago{kind}", name=f"ago{kind}"
                    )
                    nc.gpsimd.dma_start(bi[:], src_sb)
                    nc.gpsimd.collective_compute(
                        "AllGather",
                        mybir.AluOpType.bypass,
                        replica_groups=[list(range(NCORES))],
                        ins=[bi.opt()],
                        outs=[bo.opt()],
                    )
                    nc.gpsimd.dma_start(
                        dst_tile[:].rearrange("p (k w) -> p k w", k=NCORES),
                        bo[:].rearrange("(k p) w -> p k w", p=P),
                    )
                    return
                rsem = rsems[2 * kind + (t % 2)]
                nc.gpsimd.remote_dma_broadcast(
                    dst_tile[:, bass.ts(pid_pl, width)],
                    src_sb,
                    rsem,
                    lsems[2 * kind + (t % 2)],
                    rdests=RD,
                )
                nc.gpsimd.trigger_dma(count=None)
                if RSEM_PER_ROUND == 14:
                    # self slot not broadcast; copy locally
                    nc.vector.tensor_copy(
                        dst_tile[:, bass.ts(pid, width)], src_sb
                    )
                with tc.tile_critical():
                    nc.vector.wait_ge(rsem, RSEM_PER_ROUND * (t // 2 + 1))
                    nc.vector.tensor_copy(dst_tile[0:1, 0:1], dst_tile[0:1, 0:1])

            for t in range(n_steps):
                # ---- x load (fp16) + upcast ----
                xt16 = xpool.tile([P, 4, B], fp16, tag="xt16")
                nc.scalar.dma_start(xt16[:], agx[t])
                xt = xpool.tile([P, 4, B], fp32, tag="xt")
                nc.vector.tensor_copy(xt[:], xt16[:])

                # ---- gates0: K = [x(4) | o(4) | h0(8)] ----
                h0f_r = h0f[(t - 1) % 2]
                h1f_r = h1f[(t - 1) % 2]
                of_r = of[(t - 1) % 2]
                of_rv = of_r[:].rearrange("p (k c j) -> p c k j", k=NCORES, c=4)
                o4 = tpool.tile([P, 4, B], fp32, tag="o4")
                nc.vector.tensor_copy(
                    o4[:].rearrange("p c (k j) -> p c k j", k=NCORES), of_rv
                )
                pg0 = ppool.tile([P, 2, GC], fp32, tag="pg")
                order0 = [0, 1, 2, 3] + [8, 9, 10, 11, 12, 13, 14, 15] + [4, 5, 6, 7]
                for i, kt in enumerate(order0):
                    if kt < 4:
                        lhsT = xt[:, kt, :]
                    elif kt < 8:
                        lhsT = o4[:, kt - 4, :]
                    else:
                        lhsT = h0f_r[:, (kt - 8) * B : (kt - 7) * B]
                    hf = i % 2
                    nc.tensor.matmul(
                        pg0[64 * hf : 64 * hf + 64, hf, :],
                        lhsT,
                        w0T[:, kt, :],
                        start=(i < 2),
                        stop=(i >= 14),
                        tile_position=(0, 64 * hf),
                    )
                g0 = tpool.tile([B, GC], fp32, tag="g0")
                nc.vector.tensor_add(g0[:], pg0[0:64, 0, :], b0[:])
                nc.vector.tensor_add(g0[:], g0[:], pg0[64:128, 1, :])
                h0m = tpool.tile([B, HC], fp32, tag="h0m")
                lstm_pointwise(g0, c0, h0m)

                # ---- transpose h0m -> [HC, B], AG -> h0f ----
                pt0 = p1pool.tile([P, 128], fp32, tag="ptr", name="pt0")
                nc.tensor.transpose(pt0[:, 0:B], h0m[:], ident[0:B, 0:B])
                h0T = tpool.tile([P, B], fp32, tag="h0T")
                evict_src(t, 0, h0T[:], pt0[:, 0:B])
                h0src = h0T[:]

                exchange(t, 0, h0src, B, h0f[t % 2])

                # ---- gates1: K = [h0(8) | h1(8)] ----
                h0f_w = h0f[t % 2]
                pg1 = ppool.tile([P, 2, GC], fp32, tag="pg")
                order1 = [8, 9, 10, 11, 12, 13, 14, 15] + [0, 1, 2, 3, 4, 5, 6, 7]
                for i, kt in enumerate(order1):
                    lhsT = (
                        h0f_w[:, kt * B : (kt + 1) * B]
                        if kt < 8
                        else h1f_r[:, (kt - 8) * B : (kt - 7) * B]
                    )
                    hf = i % 2
                    nc.tensor.matmul(
                        pg1[64 * hf : 64 * hf + 64, hf, :],
                        lhsT,
                        w1T[:, kt, :],
                        start=(i < 2),
                        stop=(i >= 14),
                        tile_position=(0, 64 * hf),
                    )
                g1 = tpool.tile([B, GC], fp32, tag="g1")
                nc.vector.tensor_add(g1[:], pg1[0:64, 0, :], b1[:])
                nc.vector.tensor_add(g1[:], g1[:], pg1[64:128, 1, :])
                h1m = tpool.tile([B, HC], fp32, tag="h1m")
                lstm_pointwise(g1, c1, h1m)

                # ---- transpose h1m, AG -> h1f ----
                pt1 = p1pool.tile([P, 128], fp32, tag="ptr", name="pt1")
                nc.tensor.transpose(pt1[:, 0:B], h1m[:], ident[0:B, 0:B])
                h1T = tpool.tile([P, B], fp32, tag="h1T")
                evict_src(t, 1, h1T[:], pt1[:, 0:B])
                h1src = h1T[:]

                exchange(t, 1, h1src, B, h1f[t % 2])

                # ---- select my batch columns of h1 (query), fp16 for attention ----
                h1f_wv = h1f[t % 2][:].rearrange("p (kc b) -> p kc b", kc=8)
                nc.vector.tensor_copy(h1my[:], h1f_wv[:, :, bass.ts(pid, BG)])

                # ---- scores: per-b matvec via tile_position packing ----
                psc = p1pool.tile([P, 2, S], fp32, tag="psc")
                nc.vector.memset(psc[:], 0.0)
                for j in range(BG):
                    half, row = j // 4, 32 * (j % 4)
                    for kt in range(8):
                        nc.tensor.matmul(
                            psc[row : row + 1, half, :],
                            h1my[:, kt, j : j + 1],
                            m1t[:, j, kt, :],
                            start=(kt == 0),
                            stop=(kt == 7),
                            tile_position=(0, row),
                        )
                # ---- softmax over the two halves (garbage rows are fine) ----
                a_sb = tpool.tile([P, 2, S], fp32, tag="a_sb")
                stat = tpool.tile([P, 4], fp32, tag="stat")
                for half in range(2):
                    nc.vector.tensor_add(
                        a_sb[:, half, :], psc[:, half, :], c1t[:, half, :]
                    )
                    nm = stat[:, 2 * half : 2 * half + 1]
                    nc.vector.tensor_reduce(
                        nm, a_sb[:, half, :], axis=AX.X, op=mybir.AluOpType.max,
                        negate=True,
                    )
                    sm = stat[:, 2 * half + 1 : 2 * half + 2]
                    nc.scalar.activation(
                        a_sb[:, half, :], a_sb[:, half, :], AF.Exp, bias=nm,
                        accum_out=sm,
                    )
                    nc.vector.reciprocal(sm, sm)
                    nc.vector.tensor_scalar_mul(a_sb[:, half, :], a_sb[:, half, :], sm)

                # ---- transpose a -> columns; build block-diag lhsT (fp16) ----
                paT = p1pool.tile([P, 2, S], fp32, tag="psc", name="paT")
                nc.tensor.transpose(paT[:, 0, :], a_sb[:, 0, :], ident[:])
                nc.tensor.transpose(paT[:, 1, :], a_sb[:, 1, :], ident[:])
                abd = tpool.tile([P, BG * BG], fp16, tag="abd")
                nc.vector.memset(abd[:], 0.0)
                # dst cols 9j <- paT cols 128*(j//4) + 32*(j%4), one strided copy
                nc.vector.tensor_copy(
                    abd[:, 0 : BG * BG : 9].rearrange("p (a b) -> p a b", a=2),
                    paT[:].rearrange("p h (c x) -> p h c x", c=4)[:, :, :, 0:1],
                )

                # ---- z = blockdiag(a) @ M2stack + h1my.T @ W2h.T ----
                pz = p1pool.tile([BG, E], fp32, tag="pz")
                for j in range(BG):
                    nc.tensor.matmul(
                        pz[:],
                        abd[:, j * BG : (j + 1) * BG],
                        m2s[:, j, :],
                        start=(j == 0),
                        stop=False,
                    )
                for kt in range(8):
                    nc.tensor.matmul(
                        pz[:], h1my[:, kt, :], w2hT[:, kt, :], start=False,
                        stop=(kt == 7),
                    )
                zt = tpool.tile([BG, E], fp32, tag="zt")
                nc.vector.tensor_add(zt[:], pz[:], b2[:])
                o_sb = tpool.tile([BG, E], fp32, tag="o_sb")
                nc.scalar.activation(o_sb[:], zt[:], AF.Tanh)

                # ---- write output (fp16 wire) ----
                o16 = tpool.tile([BG, E], fp16, tag="o16")
                nc.vector.tensor_copy(o16[:], o_sb[:])
                nc.scalar.dma_start(d_out[t], o16[:])

                # ---- transpose o chunks -> [P, 4, BG], AG -> of ----
                poT = p1pool.tile([P, 4, BG], fp32, tag="ptr", name="poT")
                for cchunk in range(4):
                    nc.tensor.transpose(
                        poT[:, cchunk, :],
                        o_sb[:, cchunk * P : (cchunk + 1) * P],
                        ident[0:BG, 0:BG],
                    )
                oT = tpool.tile([P, 4 * BG], fp32, tag="oT")
                evict_src(t, 2, oT[:].rearrange("p (c j) -> p c j", c=4), poT[:])
                osrc = oT[:]

                exchange(t, 2, osrc, 4 * BG, of[t % 2])

    nc.compile()
    return nc


def _host_prep(inputs: dict, n_steps: int):
    """Build per-core in_maps (fp16 wire format, minimal host compute)."""
    f32, f16 = np.float32, np.float16
    tgt = np.asarray(inputs["tgt_batch"])
    h_enc = np.asarray(inputs["h_encoder"], f32)
    emb = np.asarray(inputs["emb"], f32)
    out_init = np.asarray(inputs["output_init"], f32)
    hid_init = np.asarray(inputs["hidden_init"], f32)
    W_ih = np.asarray(inputs["W_ih"], f32)
    W_hh = np.asarray(inputs["W_hh"], f32)
    b_ih = np.asarray(inputs["b_ih"], f32)
    b_hh = np.asarray(inputs["b_hh"], f32)
    W1 = np.asarray(inputs["W1"], f32)
    b1v = np.asarray(inputs["b1"], f32)
    W2 = np.asarray(inputs["W2"], f32)
    b2v = np.asarray(inputs["b2"], f32)

    npc = max(1, (n_steps + NCORES - 1) // NCORES)
    n_pad = npc * NCORES

    # x sequence, feature-major, folded [T, P, 4, B], fp16, padded to n_pad
    xs = emb[tgt[:n_steps]].astype(f16)  # [T, B, E]
    xseq = np.zeros((n_pad, P, 4, B), f16)
    xseq[:n_steps] = xs.transpose(0, 2, 1).reshape(n_steps, 4, P, B).transpose(0, 2, 1, 3)

    # full o / h inits, feature-major folds (fp16 wire)
    oi4 = out_init.T.reshape(4, P, NCORES, 8)  # [c, p, k, j]
    oi = np.ascontiguousarray(oi4.transpose(1, 2, 0, 3).reshape(P, NCORES * 4 * 8)).astype(f16)
    h0i = np.ascontiguousarray(hid_init[0].T.reshape(8, P, B).transpose(1, 0, 2)).astype(f16)
    h1i = np.ascontiguousarray(hid_init[1].T.reshape(8, P, B).transpose(1, 0, 2)).astype(f16)

    # LSTM weight shards: A[g, k, j, kt, p] view of [4096, 2048]-concat rows
    # w0T[kt, p, (g, j)] = W[g*H + k*HC + j, kt*P + p]
    Wi0 = W_ih[0].astype(f16).reshape(4, NCORES, HC, 8, P)
    Wh0 = W_hh[0].astype(f16).reshape(4, NCORES, HC, 8, P)
    Wi1 = W_ih[1].astype(f16).reshape(4, NCORES, HC, 8, P)
    Wh1 = W_hh[1].astype(f16).reshape(4, NCORES, HC, 8, P)
    bsum0 = (b_ih[0] + b_hh[0]).astype(f32).reshape(4, NCORES, HC)
    bsum1 = (b_ih[1] + b_hh[1]).astype(f32).reshape(4, NCORES, HC)

    W1_16 = W1.astype(f16)  # [V, H]
    W2T_16 = np.ascontiguousarray(W2.T).astype(f16)  # [H+V, E], rows: V then H
    h_enc16 = h_enc.astype(f16)

    in_maps = []
    for k in range(NCORES):
        # [16, P, GC]: kt<8 from W_ih, kt>=8 from W_hh
        w0T = np.empty((16, P, GC), f16)
        w0T[:8] = Wi0[:, k].transpose(2, 3, 0, 1).reshape(8, P, GC)
        w0T[8:] = Wh0[:, k].transpose(2, 3, 0, 1).reshape(8, P, GC)
        w1T = np.empty((16, P, GC), f16)
        w1T[:8] = Wi1[:, k].transpose(2, 3, 0, 1).reshape(8, P, GC)
        w1T[8:] = Wh1[:, k].transpose(2, 3, 0, 1).reshape(8, P, GC)

        biasv = np.concatenate(
            [bsum0[:, k].ravel(), bsum1[:, k].ravel(), b2v, b1v]
        ).astype(f32)[None, :]

        in_maps.append(
            {
                "xs8": xseq[k * npc:(k + 1) * npc],
                "w0T": w0T,
                "w1T": w1T,
                "henc": h_enc16[k * BG:(k + 1) * BG],
                "w1s": W1_16[k * P:(k + 1) * P],
                "w2Ts": W2T_16[k * W2R:(k + 1) * W2R],
                "biasv": biasv,
                "h0i": h0i,
                "h1i": h1i,
                "oi": oi,
            }
        )
    return in_maps


def run(inputs: dict, n_steps: int = T, trace: bool = False):
    global _COMPILED
    from concourse.bass_utils import run_bass_kernel_spmd

    if _COMPILED is None or _COMPILED[1] != n_steps:
        _COMPILED = (_build(n_steps), n_steps)
    nc = _COMPILED[0]
    in_maps = _host_prep(inputs, n_steps)
    res = run_bass_kernel_spmd(
        nc, in_maps, core_ids=list(range(NCORES)), trace=trace
    )
    outs = [res.results[k]["out"] for k in range(NCORES)]  # [T, BG, E] fp16 each
    full = np.concatenate(outs, axis=1)  # [T, B, E]
    return np.ascontiguousarray(full.transpose(1, 0, 2)).astype(np.float32), res


def kernel(**inputs) -> np.ndarray:
    out, _ = run(inputs, T)
    return out.astype(np.float32)


# revision 12
# speedup vs baseline: 5.8888x; 1.6574x over previous
"""AttentionDecoder Trainium2 kernel: 8-way model-parallel LSTM+attention decoder.

Strategy:
  - Weights sharded 8 ways over the gate/hidden dims, SBUF-resident.
  - Activations feature-major [feat, batch]; matmuls are activation-stationary
    (lhsT = activation [K=feat, M=batch], rhs = weight.T [K=feat, N=out_feats]).
  - Per timestep: 3 AllGathers (h0, h1, o) across the 8 cores.
  - Attention refactored: M1T[b] = (h_enc[b] @ W1).T and M2[b] = h_enc[b] @ W2v.T
    are precomputed ON DEVICE at the prologue (hoists h_enc out of the
    sequential loop), so per step
    scores[b] = M1T[b].T @ h1[:,b] + c1[b] and
    z[b] = a[b] @ M2[b] + W2h @ h1[:,b] + b2,  o = tanh(z).
  - Per-core batch shard for attention: core k owns batch 8k..8k+7.
  - Wire format is fp16 (the host->device tunnel is the bottleneck):
    LSTM weights upcast to fp32 on device (step-loop math unchanged),
    attention operands stay fp16 (psum accumulation is fp32).
  - Replicated arrays (xseq, W1, W2T) are sharded on the wire and
    AllGathered once on device at the prologue.
"""

import os
import warnings

warnings.filterwarnings("ignore")

import numpy as np

VOCAB, E, H, L, B, T, S, V = 32000, 512, 1024, 2, 64, 64, 128, 1024
NCORES = 8
P = 128
BG = B // NCORES  # 8 batch per core for attention
HC = H // NCORES  # 128 hidden feats per core
GC = 4 * HC  # 512 gate rows per core
W2R = (H + V) // NCORES  # 256 rows of W2T per core

REMOTE_MODE = int(os.environ.get("DEC_REMOTE", "0"))
USE_REMOTE = REMOTE_MODE >= 1

_COMPILED = None


def _build(n_steps: int):
    import concourse.bass as bass
    import concourse.bacc as bacc
    import concourse.mybir as mybir
    import concourse.tile as tile
    from concourse import masks

    fp32 = mybir.dt.float32
    fp16 = mybir.dt.float16
    AF = mybir.ActivationFunctionType
    AX = mybir.AxisListType

    npc = max(1, (n_steps + NCORES - 1) // NCORES)  # steps per core on the wire
    n_pad = npc * NCORES

    nc = bacc.Bacc(
        "TRN2",
        target_bir_lowering=False,
        debug=False,
        num_devices=NCORES,
        monotonic_sem_count=12,
    )
    rsems = [nc.monotonic_semaphore(i).sem() for i in range(6)]  # h0e,h0o,h1e,h1o,oe,oo
    lsems = [nc.monotonic_semaphore(6 + i).sem() for i in range(6)]
    import os as _os
    _rm = int(_os.environ.get("DEC_REMOTE", "0"))
    RD = [(0, d) for d in range(NCORES)]
    if _rm == 2:
        RD = [None] + [(0, d) for d in range(1, NCORES)]
    RSEM_PER_ROUND = 14 if _rm == 2 else 16

    # ---- DRAM parameters (per-core data, fp16 wire format) ----
    d_xs8 = nc.dram_tensor("xs8", [npc, P, 4, B], fp16, kind="ExternalInput")
    d_w0T = nc.dram_tensor("w0T", [16, P, GC], fp16, kind="ExternalInput")
    d_w1T = nc.dram_tensor("w1T", [16, P, GC], fp16, kind="ExternalInput")
    d_henc = nc.dram_tensor("henc", [BG, S, V], fp16, kind="ExternalInput")
    d_w1s = nc.dram_tensor("w1s", [P, H], fp16, kind="ExternalInput")
    d_w2Ts = nc.dram_tensor("w2Ts", [W2R, E], fp16, kind="ExternalInput")
    d_bias = nc.dram_tensor("biasv", [1, 2 * GC + E + V], fp32, kind="ExternalInput")
    d_h0i = nc.dram_tensor("h0i", [P, 8, B], fp16, kind="ExternalInput")
    d_h1i = nc.dram_tensor("h1i", [P, 8, B], fp16, kind="ExternalInput")
    d_oi = nc.dram_tensor("oi", [P, NCORES * 4 * BG], fp16, kind="ExternalInput")
    d_out = nc.dram_tensor("out", [n_steps, BG, E], fp16, kind="ExternalOutput")

    with tile.TileContext(nc) as tc:
        import contextlib

        ctx = contextlib.ExitStack()
        with ctx:
            wpool = ctx.enter_context(tc.tile_pool(name="weights", bufs=1))
            spool = ctx.enter_context(tc.tile_pool(name="state", bufs=1))
            xpool = ctx.enter_context(tc.tile_pool(name="x", bufs=2))
            tpool = ctx.enter_context(tc.tile_pool(name="tmp", bufs=2))
            ppool = ctx.enter_context(tc.tile_pool(name="psum", bufs=2, space="PSUM"))
            p1pool = ctx.enter_context(tc.tile_pool(name="psum1", bufs=1, space="PSUM"))
            dpool = ctx.enter_context(tc.tile_pool(name="dram", bufs=2, space="DRAM"))

            # ---- persistent SBUF tiles ----
            w0T = wpool.tile([P, 16, GC], fp32, tag="w0T")
            w1T = wpool.tile([P, 16, GC], fp32, tag="w1T")
            b0 = wpool.tile([B, GC], fp32, tag="b0")
            b1 = wpool.tile([B, GC], fp32, tag="b1")
            m1t = wpool.tile([P, BG, 8, S], fp16, tag="m1t")
            c1t = wpool.tile([P, 2, S], fp32, tag="c1t")
            m2s = wpool.tile([P, BG, E], fp16, tag="m2s")
            w2hT = wpool.tile([P, 8, E], fp16, tag="w2hT")
            b2 = wpool.tile([BG, E], fp32, tag="b2")
            ident = wpool.tile([P, P], fp32, tag="ident")
            id16 = wpool.tile([P, P], fp16, tag="id16")
            b1colf = wpool.tile([P, 8], fp32, tag="b1colf")
            b1col = wpool.tile([P, 8], fp16, tag="b1col")

            h0f = [spool.tile([P, NCORES * B], fp32, tag=f"h0f{i}", name=f"h0f{i}") for i in range(2)]
            h1f = [spool.tile([P, NCORES * B], fp32, tag=f"h1f{i}", name=f"h1f{i}") for i in range(2)]
            of = [spool.tile([P, NCORES * 4 * BG], fp32, tag=f"of{i}", name=f"of{i}") for i in range(2)]
            c0 = spool.tile([B, HC], fp32, tag="c0")  # cell states, batch-major
            c1 = spool.tile([B, HC], fp32, tag="c1")
            h1my = spool.tile([P, 8, BG], fp16, tag="h1my")

            # ---- prologue: weights (fp16 wire -> fp32 SBUF) ----
            wstage = wpool.tile([P, 16, GC], fp16, tag="wstage", name="wstage0")
            nc.sync.dma_start(wstage[:], d_w0T[:].rearrange("kt p g -> p kt g"))
            nc.vector.tensor_copy(w0T[:], wstage[:])
            wstage2 = wpool.tile([P, 16, GC], fp16, tag="wstage", name="wstage1")
            nc.sync.dma_start(wstage2[:], d_w1T[:].rearrange("kt p g -> p kt g"))
            nc.vector.tensor_copy(w1T[:], wstage2[:])

            masks.make_identity(nc, ident[:])
            nc.vector.tensor_copy(id16[:], ident[:])

            # ---- prologue: biases via K=1 outer-product broadcast ----
            bstage = wpool.tile([1, 2 * GC + E + V], fp32, tag="bstage")
            nc.sync.dma_start(bstage[:], d_bias[:])
            onesB = wpool.tile([1, B], fp32, tag="onesB")
            nc.vector.memset(onesB[:], 1.0)
            pb = ppool.tile([P, 2, GC], fp32, tag="pg", name="pb")
            nc.tensor.matmul(pb[0:B, 0, :], onesB[0:1, :], bstage[0:1, 0:GC],
                             start=True, stop=True)
            nc.vector.tensor_copy(b0[:], pb[0:B, 0, :])
            pb2 = ppool.tile([P, 2, GC], fp32, tag="pg", name="pb2")
            nc.tensor.matmul(pb2[0:B, 0, :], onesB[0:1, :], bstage[0:1, GC:2 * GC],
                             start=True, stop=True)
            nc.vector.tensor_copy(b1[:], pb2[0:B, 0, :])
            pb3 = ppool.tile([P, 2, GC], fp32, tag="pg", name="pb3")
            nc.tensor.matmul(pb3[0:BG, 0, :], onesB[0:1, 0:BG],
                             bstage[0:1, 2 * GC:2 * GC + E], start=True, stop=True)
            nc.vector.tensor_copy(b2[:], pb3[0:BG, 0, :])
            # b1 attention vector -> [P, vc] column layout, cast to fp16
            nc.sync.dma_start(
                b1colf[:], d_bias[0, 2 * GC + E:].rearrange("(vc p) -> p vc", p=P)
            )
            nc.vector.tensor_copy(b1col[:], b1colf[:])

            # ---- prologue: AllGather sharded uploads (xseq, W1, W2T) ----
            # (collectives cannot read IO tensors directly; stage via
            #  internal DRAM tiles first)
            sxs = dpool.tile([npc, P, 4, B], fp16, tag="sxs")
            nc.sync.dma_start(sxs[:], d_xs8[:])
            agx = dpool.tile([n_pad, P, 4, B], fp16, tag="agx")
            nc.gpsimd.collective_compute(
                "AllGather",
                mybir.AluOpType.bypass,
                replica_groups=[list(range(NCORES))],
                ins=[sxs.opt()],
                outs=[agx.opt()],
            )
            sw1 = dpool.tile([P, H], fp16, tag="sw1")
            nc.sync.dma_start(sw1[:], d_w1s[:])
            agw1 = dpool.tile([H, H], fp16, tag="agw1")
            nc.gpsimd.collective_compute(
                "AllGather",
                mybir.AluOpType.bypass,
                replica_groups=[list(range(NCORES))],
                ins=[sw1.opt()],
                outs=[agw1.opt()],
            )
            sw2 = dpool.tile([W2R, E], fp16, tag="sw2")
            nc.sync.dma_start(sw2[:], d_w2Ts[:])
            agw2 = dpool.tile([H + V, E], fp16, tag="agw2")
            nc.gpsimd.collective_compute(
                "AllGather",
                mybir.AluOpType.bypass,
                replica_groups=[list(range(NCORES))],
                ins=[sw2.opt()],
                outs=[agw2.opt()],
            )

            w1sb = wpool.tile([P, 8, H], fp16, tag="wstage", name="w1sb")
            nc.scalar.dma_start(w1sb[:], agw1[:].rearrange("(vc p) h -> p vc h", p=P))
            w2vT = wpool.tile([P, 8, E], fp16, tag="w2vT")
            nc.scalar.dma_start(
                w2vT[:], agw2[0:V, :].rearrange("(vc p) e -> p vc e", p=P)
            )
            nc.scalar.dma_start(
                w2hT[:], agw2[V:, :].rearrange("(hc p) e -> p hc e", p=P)
            )

            # ---- prologue: init states (fp16 wire -> fp32 SBUF) ----
            hstage = wpool.tile([P, 8, B], fp16, tag="hstage")
            nc.sync.dma_start(hstage[:], d_h0i[:])
            nc.vector.tensor_copy(
                h0f[1][:].rearrange("p (kc b) -> p kc b", kc=8), hstage[:]
            )
            hstage2 = wpool.tile([P, 8, B], fp16, tag="hstage2")
            nc.sync.dma_start(hstage2[:], d_h1i[:])
            nc.vector.tensor_copy(
                h1f[1][:].rearrange("p (kc b) -> p kc b", kc=8), hstage2[:]
            )
            ostage = wpool.tile([P, NCORES * 4 * BG], fp16, tag="ostage")
            nc.sync.dma_start(ostage[:], d_oi[:])
            nc.vector.tensor_copy(of[1][:], ostage[:])
            nc.vector.memset(c0[:], 0.0)
            nc.vector.memset(c1[:], 0.0)

            # ---- prologue: attention precompute on device ----
            # per owned batch b: m1t[b] = W1.T @ h_enc[b].T   [H, S]
            #                    m2s[b] = h_enc[b] @ W2v.T    [S, E]
            #                    c1[b]  = h_enc[b] @ b1       [S]
            nc.vector.memset(c1t[:], 0.0)
            for b in range(BG):
                hencb = tpool.tile([P, V], fp16, tag="hencb")
                nc.scalar.dma_start(hencb[:], d_henc[b])
                hencT = tpool.tile([P, 8, S], fp16, tag="hencT")
                for vc in range(8):
                    ptp = p1pool.tile([P, S], fp16, tag="ptr", name=f"ptp{b}_{vc}")
                    nc.tensor.transpose(
                        ptp[:], hencb[:, vc * P:(vc + 1) * P], id16[:]
                    )
                    nc.vector.tensor_copy(hencT[:, vc, :], ptp[:])
                # m1t: 8 h-chunks, accumulate over 8 v-chunks
                for hc in range(8):
                    pm = p1pool.tile([P, S], fp32, tag="ptr", name=f"pm{b}_{hc}")
                    for vc in range(8):
                        nc.tensor.matmul(
                            pm[:],
                            w1sb[:, vc, hc * P:(hc + 1) * P],
                            hencT[:, vc, :],
                            start=(vc == 0),
                            stop=(vc == 7),
                        )
                    nc.vector.tensor_copy(m1t[:, b, hc, :], pm[:])
                # m2s: accumulate over 8 v-chunks
                pm2 = p1pool.tile([P, E], fp32, tag="pz", name=f"pm2{b}")
                for vc in range(8):
                    nc.tensor.matmul(
                        pm2[:],
                        hencT[:, vc, :],
                        w2vT[:, vc, :],
                        start=(vc == 0),
                        stop=(vc == 7),
                    )
                nc.vector.tensor_copy(m2s[:, b, :], pm2[:])
                # c1 row: single-row matvec into partition 32*(b%4), half b//4
                half, row = b // 4, 32 * (b % 4)
                pc1 = p1pool.tile([P, 2, S], fp32, tag="psc", name=f"pc1{b}")
                for vc in range(8):
                    nc.tensor.matmul(
                        pc1[row:row + 1, half, :],
                        b1col[:, vc:vc + 1],
                        hencT[:, vc, :],
                        start=(vc == 0),
                        stop=(vc == 7),
                        tile_position=(0, row),
                    )
                nc.vector.tensor_copy(
                    c1t[row:row + 1, half, :], pc1[row:row + 1, half, :]
                )

            pid = nc.vector.partition_id()
            pid_pl = nc.gpsimd.partition_id()

            def lstm_pointwise(g_sb, cst, h_out):
                """g_sb [B, 4*HC] gate order i,f,g,o; updates cst, writes h_out [B,HC]."""
                gt = tpool.tile([B, HC], fp32, tag="pw_gt")
                ot = tpool.tile([B, HC], fp32, tag="pw_ot")
                ift = tpool.tile([B, 2 * HC], fp32, tag="pw_ift")
                nc.scalar.activation(ift[:], g_sb[:, 0 : 2 * HC], AF.Sigmoid)
                it, ft = ift[:, 0:HC], ift[:, HC : 2 * HC]
                nc.scalar.activation(gt[:], g_sb[:, 2 * HC : 3 * HC], AF.Tanh)
                nc.scalar.activation(ot[:], g_sb[:, 3 * HC : 4 * HC], AF.Sigmoid)
                t1 = tpool.tile([B, HC], fp32, tag="pw_t1")
                nc.vector.tensor_mul(t1[:], ft, cst[:])
                nc.vector.tensor_mul(gt[:], it, gt[:])
                nc.vector.tensor_add(cst[:], t1[:], gt[:])
                tc_ = tpool.tile([B, HC], fp32, tag="pw_tc")
                nc.scalar.activation(tc_[:], cst[:], AF.Tanh)
                nc.vector.tensor_mul(h_out[:], ot[:], tc_[:])

            def evict_src(t, kind, dst_ap, src_ap):
                if t >= 2 and USE_REMOTE:
                    with tc.tile_critical():
                        nc.vector.wait_ge(lsems[2 * kind + (t % 2)], 16 * (t // 2))
                        nc.vector.tensor_copy(dst_ap, src_ap)
                else:
                    nc.vector.tensor_copy(dst_ap, src_ap)

            def exchange(t, kind, src_sb, width, dst_tile):
                """Broadcast my [P,width] chunk into slot pid of everyone's dst_tile."""
                if not USE_REMOTE:
                    bi = dpool.tile([P, width], fp32, tag=f"agi{kind}", name=f"agi{kind}")
                    bo = dpool.tile(
                        [P * NCORES, width], fp32, tag=f"ago{kind}", name=f"ago{kind}"
                    )
                    nc.gpsimd.dma_start(bi[:], src_sb)
                    nc.gpsimd.collective_compute(
                        "AllGather",
                        mybir.AluOpType.bypass,
                        replica_groups=[list(range(NCORES))],
                        ins=[bi.opt()],
                        outs=[bo.opt()],
                    )
                    nc.gpsimd.dma_start(
                        dst_tile[:].rearrange("p (k w) -> p k w", k=NCORES),
                        bo[:].rearrange("(k p) w -> p k w", p=P),
                    )
                    return
                rsem = rsems[2 * kind + (t % 2)]
                nc.gpsimd.remote_dma_broadcast(
                    dst_tile[:, bass.ts(pid_pl, width)],
                    src_sb,
                    rsem,
                    lsems[2 * kind + (t % 2)],
                    rdests=RD,
                )
                nc.gpsimd.trigger_dma(count=None)
                if RSEM_PER_ROUND == 14:
                    # self slot not broadcast; copy locally
                    nc.vector.tensor_copy(
                        dst_tile[:, bass.ts(pid, width)], src_sb
                    )
                with tc.tile_critical():
                    nc.vector.wait_ge(rsem, RSEM_PER_ROUND * (t // 2 + 1))
                    nc.vector.tensor_copy(dst_tile[0:1, 0:1], dst_tile[0:1, 0:1])

            for t in range(n_steps):
                # ---- x load (fp16) + upcast ----
                xt16 = xpool.tile([P, 4, B], fp16, tag="xt16")
                nc.scalar.dma_start(xt16[:], agx[t])
                xt = xpool.tile([P, 4, B], fp32, tag="xt")
                nc.vector.tensor_copy(xt[:], xt16[:])

                # ---- gates0: K = [x(4) | o(4) | h0(8)] ----
                h0f_r = h0f[(t - 1) % 2]
                h1f_r = h1f[(t - 1) % 2]
                of_r = of[(t - 1) % 2]
                of_rv = of_r[:].rearrange("p (k c j) -> p c k j", k=NCORES, c=4)
                o4 = tpool.tile([P, 4, B], fp32, tag="o4")
                nc.vector.tensor_copy(
                    o4[:].rearrange("p c (k j) -> p c k j", k=NCORES), of_rv
                )
                pg0 = ppool.tile([P, 2, GC], fp32, tag="pg")
                order0 = [0, 1, 2, 3] + [8, 9, 10, 11, 12, 13, 14, 15] + [4, 5, 6, 7]
                for i, kt in enumerate(order0):
                    if kt < 4:
                        lhsT = xt[:, kt, :]
                    elif kt < 8:
                        lhsT = o4[:, kt - 4, :]
                    else:
                        lhsT = h0f_r[:, (kt - 8) * B : (kt - 7) * B]
                    hf = i % 2
                    nc.tensor.matmul(
                        pg0[64 * hf : 64 * hf + 64, hf, :],
                        lhsT,
                        w0T[:, kt, :],
                        start=(i < 2),
                        stop=(i >= 14),
                        tile_position=(0, 64 * hf),
                    )
                g0 = tpool.tile([B, GC], fp32, tag="g0")
                nc.vector.tensor_add(g0[:], pg0[0:64, 0, :], b0[:])
                nc.vector.tensor_add(g0[:], g0[:], pg0[64:128, 1, :])
                h0m = tpool.tile([B, HC], fp32, tag="h0m")
                lstm_pointwise(g0, c0, h0m)

                # ---- transpose h0m -> [HC, B], AG -> h0f ----
                pt0 = p1pool.tile([P, 128], fp32, tag="ptr", name="pt0")
                nc.tensor.transpose(pt0[:, 0:B], h0m[:], ident[0:B, 0:B])
                h0T = tpool.tile([P, B], fp32, tag="h0T")
                evict_src(t, 0, h0T[:], pt0[:, 0:B])
                h0src = h0T[:]

                exchange(t, 0, h0src, B, h0f[t % 2])

                # ---- gates1: K = [h0(8) | h1(8)] ----
                h0f_w = h0f[t % 2]
                pg1 = ppool.tile([P, 2, GC], fp32, tag="pg")
                order1 = [8, 9, 10, 11, 12, 13, 14, 15] + [0, 1, 2, 3, 4, 5, 6, 7]
                for i, kt in enumerate(order1):
                    lhsT = (
                        h0f_w[:, kt * B : (kt + 1) * B]
                        if kt < 8
                        else h1f_r[:, (kt - 8) * B : (kt - 7) * B]
                    )
                    hf = i % 2
                    nc.tensor.matmul(
                        pg1[64 * hf : 64 * hf + 64, hf, :],
                        lhsT,
                        w1T[:, kt, :],
                        start=(i < 2),
                        stop=(i >= 14),
                        tile_position=(0, 64 * hf),
                    )
                g1 = tpool.tile([B, GC], fp32, tag="g1")
                nc.vector.tensor_add(g1[:], pg1[0:64, 0, :], b1[:])
                nc.vector.tensor_add(g1[:], g1[:], pg1[64:128, 1, :])
                h1m = tpool.tile([B, HC], fp32, tag="h1m")
                lstm_pointwise(g1, c1, h1m)

                # ---- transpose h1m, AG -> h1f ----
                pt1 = p1pool.tile([P, 128], fp32, tag="ptr", name="pt1")
                nc.tensor.transpose(pt1[:, 0:B], h1m[:], ident[0:B, 0:B])
                h1T = tpool.tile([P, B], fp32, tag="h1T")
                evict_src(t, 1, h1T[:], pt1[:, 0:B])
                h1src = h1T[:]

                exchange(t, 1, h1src, B, h1f[t % 2])

                # ---- select my batch columns of h1 (query), fp16 for attention ----
                h1f_wv = h1f[t % 2][:].rearrange("p (kc b) -> p kc b", kc=8)
                nc.vector.tensor_copy(h1my[:], h1f_wv[:, :, bass.ts(pid, BG)])

                # ---- scores: per-b matvec via tile_position packing ----
                psc = p1pool.tile([P, 2, S], fp32, tag="psc")
                nc.vector.memset(psc[:], 0.0)
                for j in range(BG):
                    half, row = j // 4, 32 * (j % 4)
                    for kt in range(8):
                        nc.tensor.matmul(
                            psc[row : row + 1, half, :],
                            h1my[:, kt, j : j + 1],
                            m1t[:, j, kt, :],
                            start=(kt == 0),
                            stop=(kt == 7),
                            tile_position=(0, row),
                        )
                # ---- softmax over the two halves (garbage rows are fine) ----
                a_sb = tpool.tile([P, 2, S], fp32, tag="a_sb")
                stat = tpool.tile([P, 4], fp32, tag="stat")
                for half in range(2):
                    nc.vector.tensor_add(
                        a_sb[:, half, :], psc[:, half, :], c1t[:, half, :]
                    )
                    nm = stat[:, 2 * half : 2 * half + 1]
                    nc.vector.tensor_reduce(
                        nm, a_sb[:, half, :], axis=AX.X, op=mybir.AluOpType.max,
                        negate=True,
                    )
                    sm = stat[:, 2 * half + 1 : 2 * half + 2]
                    nc.scalar.activation(
                        a_sb[:, half, :], a_sb[:, half, :], AF.Exp, bias=nm,
                        accum_out=sm,
                    )
                    nc.vector.reciprocal(sm, sm)
                    nc.vector.tensor_scalar_mul(a_sb[:, half, :], a_sb[:, half, :], sm)

                # ---- transpose a -> columns; build block-diag lhsT (fp16) ----
                paT = p1pool.tile([P, 2, S], fp32, tag="psc", name="paT")
                nc.tensor.transpose(paT[:, 0, :], a_sb[:, 0, :], ident[:])
                nc.tensor.transpose(paT[:, 1, :], a_sb[:, 1, :], ident[:])
                abd = tpool.tile([P, BG * BG], fp16, tag="abd")
                nc.vector.memset(abd[:], 0.0)
                # dst cols 9j <- paT cols 128*(j//4) + 32*(j%4), one strided copy
                nc.vector.tensor_copy(
                    abd[:, 0 : BG * BG : 9].rearrange("p (a b) -> p a b", a=2),
                    paT[:].rearrange("p h (c x) -> p h c x", c=4)[:, :, :, 0:1],
                )

                # ---- z = blockdiag(a) @ M2stack + h1my.T @ W2h.T ----
                pz = p1pool.tile([BG, E], fp32, tag="pz")
                for j in range(BG):
                    nc.tensor.matmul(
                        pz[:],
                        abd[:, j * BG : (j + 1) * BG],
                        m2s[:, j, :],
                        start=(j == 0),
                        stop=False,
                    )
                for kt in range(8):
                    nc.tensor.matmul(
                        pz[:], h1my[:, kt, :], w2hT[:, kt, :], start=False,
                        stop=(kt == 7),
                    )
                zt = tpool.tile([BG, E], fp32, tag="zt")
                nc.vector.tensor_add(zt[:], pz[:], b2[:])
                o_sb = tpool.tile([BG, E], fp32, tag="o_sb")
                nc.scalar.activation(o_sb[:], zt[:], AF.Tanh)

                # ---- write output (fp16 wire) ----
                o16 = tpool.tile([BG, E], fp16, tag="o16")
                nc.vector.tensor_copy(o16[:], o_sb[:])
                nc.scalar.dma_start(d_out[t], o16[:])

                # ---- transpose o chunks -> [P, 4, BG], AG -> of ----
                poT = p1pool.tile([P, 4, BG], fp32, tag="ptr", name="poT")
                for cchunk in range(4):
                    nc.tensor.transpose(
                        poT[:, cchunk, :],
                        o_sb[:, cchunk * P : (cchunk + 1) * P],
                        ident[0:BG, 0:BG],
                    )
                oT = tpool.tile([P, 4 * BG], fp32, tag="oT")
                evict_src(t, 2, oT[:].rearrange("p (c j) -> p c j", c=4), poT[:])
                osrc = oT[:]

                exchange(t, 2, osrc, 4 * BG, of[t % 2])

    nc.compile()
    return nc


def _host_prep(inputs: dict, n_steps: int):
    """Build per-core in_maps (fp16 wire format, minimal host compute)."""
    f32, f16 = np.float32, np.float16
    tgt = np.asarray(inputs["tgt_batch"])
    h_enc = np.asarray(inputs["h_encoder"], f32)
    emb = np.asarray(inputs["emb"], f32)
    out_init = np.asarray(inputs["output_init"], f32)
    hid_init = np.asarray(inputs["hidden_init"], f32)
    W_ih = np.asarray(inputs["W_ih"], f32)
    W_hh = np.asarray(inputs["W_hh"], f32)
    b_ih = np.asarray(inputs["b_ih"], f32)
    b_hh = np.asarray(inputs["b_hh"], f32)
    W1 = np.asarray(inputs["W1"], f32)
    b1v = np.asarray(inputs["b1"], f32)
    W2 = np.asarray(inputs["W2"], f32)
    b2v = np.asarray(inputs["b2"], f32)

    npc = max(1, (n_steps + NCORES - 1) // NCORES)
    n_pad = npc * NCORES

    # x sequence, feature-major, folded [T, P, 4, B], fp16, padded to n_pad
    xs = emb[tgt[:n_steps]].astype(f16)  # [T, B, E]
    xseq = np.zeros((n_pad, P, 4, B), f16)
    xseq[:n_steps] = xs.transpose(0, 2, 1).reshape(n_steps, 4, P, B).transpose(0, 2, 1, 3)

    # full o / h inits, feature-major folds (fp16 wire)
    oi4 = out_init.T.reshape(4, P, NCORES, 8)  # [c, p, k, j]
    oi = np.ascontiguousarray(oi4.transpose(1, 2, 0, 3).reshape(P, NCORES * 4 * 8)).astype(f16)
    h0i = np.ascontiguousarray(hid_init[0].T.reshape(8, P, B).transpose(1, 0, 2)).astype(f16)
    h1i = np.ascontiguousarray(hid_init[1].T.reshape(8, P, B).transpose(1, 0, 2)).astype(f16)

    # LSTM weight shards: A[g, k, j, kt, p] view of [4096, 2048]-concat rows
    # w0T[kt, p, (g, j)] = W[g*H + k*HC + j, kt*P + p]
    Wi0 = W_ih[0].astype(f16).reshape(4, NCORES, HC, 8, P)
    Wh0 = W_hh[0].astype(f16).reshape(4, NCORES, HC, 8, P)
    Wi1 = W_ih[1].astype(f16).reshape(4, NCORES, HC, 8, P)
    Wh1 = W_hh[1].astype(f16).reshape(4, NCORES, HC, 8, P)
    bsum0 = (b_ih[0] + b_hh[0]).astype(f32).reshape(4, NCORES, HC)
    bsum1 = (b_ih[1] + b_hh[1]).astype(f32).reshape(4, NCORES, HC)

    W1_16 = W1.astype(f16)  # [V, H]
    W2T_16 = np.ascontiguousarray(W2.T).astype(f16)  # [H+V, E], rows: V then H
    h_enc16 = h_enc.astype(f16)

    in_maps = []
    for k in range(NCORES):
        # [16, P, GC]: kt<8 from W_ih, kt>=8 from W_hh
        w0T = np.empty((16, P, GC), f16)
        w0T[:8] = Wi0[:, k].transpose(2, 3, 0, 1).reshape(8, P, GC)
        w0T[8:] = Wh0[:, k].transpose(2, 3, 0, 1).reshape(8, P, GC)
        w1T = np.empty((16, P, GC), f16)
        w1T[:8] = Wi1[:, k].transpose(2, 3, 0, 1).reshape(8, P, GC)
        w1T[8:] = Wh1[:, k].transpose(2, 3, 0, 1).reshape(8, P, GC)

        biasv = np.concatenate(
            [bsum0[:, k].ravel(), bsum1[:, k].ravel(), b2v, b1v]
        ).astype(f32)[None, :]

        in_maps.append(
            {
                "xs8": xseq[k * npc:(k + 1) * npc],
                "w0T": w0T,
                "w1T": w1T,
                "henc": h_enc16[k * BG:(k + 1) * BG],
                "w1s": W1_16[k * P:(k + 1) * P],
                "w2Ts": W2T_16[k * W2R:(k + 1) * W2R],
                "biasv": biasv,
                "h0i": h0i,
                "h1i": h1i,
                "oi": oi,
            }
        )
    return in_maps


_DISPATCH = None


def _make_dispatch(nc):
    """Cached PJRT dispatch (same route as bass2jax.run_bass_via_pjrt, but the
    jax.jit wrapper is built once so steady-state calls skip re-tracing, and
    the zero output buffers are created on device instead of shipped)."""
    import jax
    from jax.sharding import Mesh, PartitionSpec
    from jax.experimental.shard_map import shard_map
    import concourse.bass2jax as b2j
    import concourse.mybir as mybir

    b2j.install_neuronx_cc_hook()
    partition_name = nc.partition_id_tensor.name if nc.partition_id_tensor else None
    in_names, out_names, out_avals = [], [], []
    for alloc in nc.m.functions[0].allocations:
        if not isinstance(alloc, mybir.MemoryLocationSet):
            continue
        name = alloc.memorylocations[0].name
        if alloc.kind == "ExternalInput":
            if name != partition_name:
                in_names.append(name)
        elif alloc.kind == "ExternalOutput":
            out_names.append(name)
            shape = tuple(alloc.tensor_shape)
            dtype = mybir.dt.np(alloc.dtype)
            out_avals.append(jax.core.ShapedArray(shape, dtype))
    n_params = len(in_names)
    all_names = list(in_names) + out_names
    if partition_name is not None:
        all_names.append(partition_name)

    import jax.numpy as jnp

    n_outs = len(out_avals)

    def _body(*args):
        operands = list(args)
        if partition_name is not None:
            operands.append(b2j.partition_id_tensor())
        return tuple(
            b2j._bass_exec_p.bind(
                *operands,
                out_avals=tuple(out_avals),
                in_names=tuple(all_names),
                out_names=tuple(out_names),
                lowering_input_output_aliases=(),
                sim_require_finite=True,
                sim_require_nnan=True,
                nc=nc,
            )
        )

    devices = jax.devices()[:NCORES]
    mesh = Mesh(np.asarray(devices), ("core",))
    sharded = jax.jit(
        shard_map(
            _body,
            mesh=mesh,
            in_specs=(PartitionSpec("core"),) * (n_params + n_outs),
            out_specs=(PartitionSpec("core"),) * len(out_names),
            check_rep=False,
        ),
        donate_argnums=tuple(range(n_params, n_params + n_outs)),
        keep_unused=True,
    )

    # donated zero output buffers, made on device (nothing crosses the wire)
    from jax.sharding import NamedSharding

    zshapes = [
        ((NCORES * a.shape[0], *a.shape[1:]), a.dtype) for a in out_avals
    ]
    zeros_maker = jax.jit(
        lambda: tuple(jnp.zeros(s, d) for s, d in zshapes),
        out_shardings=tuple(
            NamedSharding(mesh, PartitionSpec("core")) for _ in zshapes
        ),
    )
    return sharded, in_names, out_names, out_avals, zeros_maker


def run(inputs: dict, n_steps: int = T, trace: bool = False):
    global _COMPILED, _DISPATCH
    if _COMPILED is None or _COMPILED[1] != n_steps:
        _COMPILED = (_build(n_steps), n_steps)
        _DISPATCH = None
    nc = _COMPILED[0]
    in_maps = _host_prep(inputs, n_steps)
    if _DISPATCH is None:
        _DISPATCH = _make_dispatch(nc)
    sharded, in_names, out_names, out_avals, zeros_maker = _DISPATCH
    zeros = zeros_maker()
    concat_in = [
        np.concatenate([np.asarray(in_maps[c][nm]) for c in range(NCORES)], axis=0)
        for nm in in_names
    ]
    out_arrs = sharded(*concat_in, *zeros)
    oidx = out_names.index("out")
    full_out = np.asarray(out_arrs[oidx]).reshape(
        NCORES, *out_avals[oidx].shape
    )  # [NCORES, T, BG, E] fp16
    full = np.concatenate(list(full_out), axis=1)  # [T, B, E]
    return np.ascontiguousarray(full.transpose(1, 0, 2)).astype(np.float32), None


def kernel(**inputs) -> np.ndarray:
    out, _ = run(inputs, T)
    return out.astype(np.float32)


# revision 17
# speedup vs baseline: 6.6457x; 1.1285x over previous
"""AttentionDecoder Trainium2 kernel: 8-way model-parallel LSTM+attention decoder.

Strategy:
  - Weights sharded 8 ways over the gate/hidden dims, SBUF-resident.
  - Activations feature-major [feat, batch]; matmuls are activation-stationary
    (lhsT = activation [K=feat, M=batch], rhs = weight.T [K=feat, N=out_feats]).
  - Per timestep: 3 AllGathers (h0, h1, o) across the 8 cores.
  - Attention refactored: M1T[b] = (h_enc[b] @ W1).T and M2[b] = h_enc[b] @ W2v.T
    are precomputed ON DEVICE at the prologue (hoists h_enc out of the
    sequential loop), so per step
    scores[b] = M1T[b].T @ h1[:,b] + c1[b] and
    z[b] = a[b] @ M2[b] + W2h @ h1[:,b] + b2,  o = tanh(z).
  - Per-core batch shard for attention: core k owns batch 8k..8k+7.
  - Wire format is fp16 (the host->device tunnel is the bottleneck):
    LSTM weights upcast to fp32 on device (step-loop math unchanged),
    attention operands stay fp16 (psum accumulation is fp32).
  - Replicated arrays (xseq, W1, W2T) are sharded on the wire and
    AllGathered once on device at the prologue.
"""

import os
import warnings

warnings.filterwarnings("ignore")

import numpy as np

VOCAB, E, H, L, B, T, S, V = 32000, 512, 1024, 2, 64, 64, 128, 1024
NCORES = 8
P = 128
BG = B // NCORES  # 8 batch per core for attention
HC = H // NCORES  # 128 hidden feats per core
GC = 4 * HC  # 512 gate rows per core
W2R = (H + V) // NCORES  # 256 rows of W2T per core

REMOTE_MODE = int(os.environ.get("DEC_REMOTE", "0"))
USE_REMOTE = REMOTE_MODE >= 1

_COMPILED = None


def _build(n_steps: int):
    import concourse.bass as bass
    import concourse.bacc as bacc
    import concourse.mybir as mybir
    import concourse.tile as tile
    from concourse import masks

    fp32 = mybir.dt.float32
    fp16 = mybir.dt.float16
    AF = mybir.ActivationFunctionType
    AX = mybir.AxisListType

    npc = max(1, (n_steps + NCORES - 1) // NCORES)  # steps per core on the wire
    n_pad = npc * NCORES

    nc = bacc.Bacc(
        "TRN2",
        target_bir_lowering=False,
        debug=False,
        num_devices=NCORES,
        monotonic_sem_count=12,
    )
    rsems = [nc.monotonic_semaphore(i).sem() for i in range(6)]  # h0e,h0o,h1e,h1o,oe,oo
    lsems = [nc.monotonic_semaphore(6 + i).sem() for i in range(6)]
    import os as _os
    _rm = int(_os.environ.get("DEC_REMOTE", "0"))
    RD = [(0, d) for d in range(NCORES)]
    if _rm == 2:
        RD = [None] + [(0, d) for d in range(1, NCORES)]
    RSEM_PER_ROUND = 14 if _rm == 2 else 16

    # ---- DRAM parameters (per-core data, fp16 wire format) ----
    d_xs8 = nc.dram_tensor("xs8", [npc, P, 4, B], fp16, kind="ExternalInput")
    d_w0T = nc.dram_tensor("w0T", [16, P, GC], fp16, kind="ExternalInput")
    d_w1T = nc.dram_tensor("w1T", [16, P, GC], fp16, kind="ExternalInput")
    d_henc = nc.dram_tensor("henc", [BG, S, V], fp16, kind="ExternalInput")
    d_w1s = nc.dram_tensor("w1s", [P, H], fp16, kind="ExternalInput")
    d_w2Ts = nc.dram_tensor("w2Ts", [W2R, E], fp16, kind="ExternalInput")
    d_bias = nc.dram_tensor("biasv", [1, 2 * GC + E + V], fp32, kind="ExternalInput")
    d_h0i = nc.dram_tensor("h0i", [P, 8, B], fp16, kind="ExternalInput")
    d_h1i = nc.dram_tensor("h1i", [P, 8, B], fp16, kind="ExternalInput")
    d_oi = nc.dram_tensor("oi", [P, NCORES * 4 * BG], fp16, kind="ExternalInput")
    d_out = nc.dram_tensor("out", [BG, n_steps, E], fp16, kind="ExternalOutput")

    with tile.TileContext(nc) as tc:
        import contextlib

        ctx = contextlib.ExitStack()
        with ctx:
            wpool = ctx.enter_context(tc.tile_pool(name="weights", bufs=1))
            spool = ctx.enter_context(tc.tile_pool(name="state", bufs=1))
            xpool = ctx.enter_context(tc.tile_pool(name="x", bufs=2))
            tpool = ctx.enter_context(tc.tile_pool(name="tmp", bufs=2))
            ppool = ctx.enter_context(tc.tile_pool(name="psum", bufs=2, space="PSUM"))
            p1pool = ctx.enter_context(tc.tile_pool(name="psum1", bufs=1, space="PSUM"))
            dpool = ctx.enter_context(tc.tile_pool(name="dram", bufs=2, space="DRAM"))

            # ---- persistent SBUF tiles ----
            w0T = wpool.tile([P, 16, GC], fp32, tag="w0T")
            w1T = wpool.tile([P, 16, GC], fp32, tag="w1T")
            b0 = wpool.tile([B, GC], fp32, tag="b0")
            b1 = wpool.tile([B, GC], fp32, tag="b1")
            m1t = wpool.tile([P, BG, 8, S], fp16, tag="m1t")
            c1t = wpool.tile([P, 2, S], fp32, tag="c1t")
            m2s = wpool.tile([P, BG, E], fp16, tag="m2s")
            w2hT = wpool.tile([P, 8, E], fp16, tag="w2hT")
            b2 = wpool.tile([BG, E], fp32, tag="b2")
            ident = wpool.tile([P, P], fp32, tag="ident")
            id16 = wpool.tile([P, P], fp16, tag="id16")
            b1colf = wpool.tile([P, 8], fp32, tag="b1colf")
            b1col = wpool.tile([P, 8], fp16, tag="b1col")

            h0f = [spool.tile([P, NCORES * B], fp32, tag=f"h0f{i}", name=f"h0f{i}") for i in range(2)]
            h1f = [spool.tile([P, NCORES * B], fp32, tag=f"h1f{i}", name=f"h1f{i}") for i in range(2)]
            of = [spool.tile([P, NCORES * 4 * BG], fp32, tag=f"of{i}", name=f"of{i}") for i in range(2)]
            c0 = spool.tile([B, HC], fp32, tag="c0")  # cell states, batch-major
            c1 = spool.tile([B, HC], fp32, tag="c1")
            h1my = spool.tile([P, 8, BG], fp16, tag="h1my")

            # ---- prologue: weights (fp16 wire -> fp32 SBUF) ----
            wstage = wpool.tile([P, 16, GC], fp16, tag="wstage", name="wstage0")
            nc.sync.dma_start(wstage[:], d_w0T[:].rearrange("kt p g -> p kt g"))
            nc.vector.tensor_copy(w0T[:], wstage[:])
            wstage2 = wpool.tile([P, 16, GC], fp16, tag="wstage", name="wstage1")
            nc.sync.dma_start(wstage2[:], d_w1T[:].rearrange("kt p g -> p kt g"))
            nc.vector.tensor_copy(w1T[:], wstage2[:])

            masks.make_identity(nc, ident[:])
            nc.vector.tensor_copy(id16[:], ident[:])

            # ---- prologue: biases via K=1 outer-product broadcast ----
            bstage = wpool.tile([1, 2 * GC + E + V], fp32, tag="bstage")
            nc.sync.dma_start(bstage[:], d_bias[:])
            onesB = wpool.tile([1, B], fp32, tag="onesB")
            nc.vector.memset(onesB[:], 1.0)
            pb = ppool.tile([P, 2, GC], fp32, tag="pg", name="pb")
            nc.tensor.matmul(pb[0:B, 0, :], onesB[0:1, :], bstage[0:1, 0:GC],
                             start=True, stop=True)
            nc.vector.tensor_copy(b0[:], pb[0:B, 0, :])
            pb2 = ppool.tile([P, 2, GC], fp32, tag="pg", name="pb2")
            nc.tensor.matmul(pb2[0:B, 0, :], onesB[0:1, :], bstage[0:1, GC:2 * GC],
                             start=True, stop=True)
            nc.vector.tensor_copy(b1[:], pb2[0:B, 0, :])
            pb3 = ppool.tile([P, 2, GC], fp32, tag="pg", name="pb3")
            nc.tensor.matmul(pb3[0:BG, 0, :], onesB[0:1, 0:BG],
                             bstage[0:1, 2 * GC:2 * GC + E], start=True, stop=True)
            nc.vector.tensor_copy(b2[:], pb3[0:BG, 0, :])
            # b1 attention vector -> [P, vc] column layout, cast to fp16
            nc.sync.dma_start(
                b1colf[:], d_bias[0, 2 * GC + E:].rearrange("(vc p) -> p vc", p=P)
            )
            nc.vector.tensor_copy(b1col[:], b1colf[:])

            # ---- prologue: AllGather sharded uploads (xseq, W1, W2T) ----
            # (collectives cannot read IO tensors directly; stage via
            #  internal DRAM tiles first)
            sxs = dpool.tile([npc, P, 4, B], fp16, tag="sxs")
            nc.sync.dma_start(sxs[:], d_xs8[:])
            agx = dpool.tile([n_pad, P, 4, B], fp16, tag="agx")
            nc.gpsimd.collective_compute(
                "AllGather",
                mybir.AluOpType.bypass,
                replica_groups=[list(range(NCORES))],
                ins=[sxs.opt()],
                outs=[agx.opt()],
            )
            sw1 = dpool.tile([P, H], fp16, tag="sw1")
            nc.sync.dma_start(sw1[:], d_w1s[:])
            agw1 = dpool.tile([H, H], fp16, tag="agw1")
            nc.gpsimd.collective_compute(
                "AllGather",
                mybir.AluOpType.bypass,
                replica_groups=[list(range(NCORES))],
                ins=[sw1.opt()],
                outs=[agw1.opt()],
            )
            sw2 = dpool.tile([W2R, E], fp16, tag="sw2")
            nc.sync.dma_start(sw2[:], d_w2Ts[:])
            agw2 = dpool.tile([H + V, E], fp16, tag="agw2")
            nc.gpsimd.collective_compute(
                "AllGather",
                mybir.AluOpType.bypass,
                replica_groups=[list(range(NCORES))],
                ins=[sw2.opt()],
                outs=[agw2.opt()],
            )

            w1sb = wpool.tile([P, 8, H], fp16, tag="wstage", name="w1sb")
            nc.scalar.dma_start(w1sb[:], agw1[:].rearrange("(vc p) h -> p vc h", p=P))
            w2vT = wpool.tile([P, 8, E], fp16, tag="w2vT")
            nc.scalar.dma_start(
                w2vT[:], agw2[0:V, :].rearrange("(vc p) e -> p vc e", p=P)
            )
            nc.scalar.dma_start(
                w2hT[:], agw2[V:, :].rearrange("(hc p) e -> p hc e", p=P)
            )

            # ---- prologue: init states (fp16 wire -> fp32 SBUF) ----
            hstage = wpool.tile([P, 8, B], fp16, tag="hstage")
            nc.sync.dma_start(hstage[:], d_h0i[:])
            nc.vector.tensor_copy(
                h0f[1][:].rearrange("p (kc b) -> p kc b", kc=8), hstage[:]
            )
            hstage2 = wpool.tile([P, 8, B], fp16, tag="hstage2")
            nc.sync.dma_start(hstage2[:], d_h1i[:])
            nc.vector.tensor_copy(
                h1f[1][:].rearrange("p (kc b) -> p kc b", kc=8), hstage2[:]
            )
            ostage = wpool.tile([P, NCORES * 4 * BG], fp16, tag="ostage")
            nc.sync.dma_start(ostage[:], d_oi[:])
            nc.vector.tensor_copy(of[1][:], ostage[:])
            nc.vector.memset(c0[:], 0.0)
            nc.vector.memset(c1[:], 0.0)

            # ---- prologue: attention precompute on device ----
            # per owned batch b: m1t[b] = W1.T @ h_enc[b].T   [H, S]
            #                    m2s[b] = h_enc[b] @ W2v.T    [S, E]
            #                    c1[b]  = h_enc[b] @ b1       [S]
            nc.vector.memset(c1t[:], 0.0)
            for b in range(BG):
                hencb = tpool.tile([P, V], fp16, tag="hencb")
                nc.scalar.dma_start(hencb[:], d_henc[b])
                hencT = tpool.tile([P, 8, S], fp16, tag="hencT")
                for vc in range(8):
                    ptp = p1pool.tile([P, S], fp16, tag="ptr", name=f"ptp{b}_{vc}")
                    nc.tensor.transpose(
                        ptp[:], hencb[:, vc * P:(vc + 1) * P], id16[:]
                    )
                    nc.vector.tensor_copy(hencT[:, vc, :], ptp[:])
                # m1t: 8 h-chunks, accumulate over 8 v-chunks
                for hc in range(8):
                    pm = p1pool.tile([P, S], fp32, tag="ptr", name=f"pm{b}_{hc}")
                    for vc in range(8):
                        nc.tensor.matmul(
                            pm[:],
                            w1sb[:, vc, hc * P:(hc + 1) * P],
                            hencT[:, vc, :],
                            start=(vc == 0),
                            stop=(vc == 7),
                        )
                    nc.vector.tensor_copy(m1t[:, b, hc, :], pm[:])
                # m2s: accumulate over 8 v-chunks
                pm2 = p1pool.tile([P, E], fp32, tag="pz", name=f"pm2{b}")
                for vc in range(8):
                    nc.tensor.matmul(
                        pm2[:],
                        hencT[:, vc, :],
                        w2vT[:, vc, :],
                        start=(vc == 0),
                        stop=(vc == 7),
                    )
                nc.vector.tensor_copy(m2s[:, b, :], pm2[:])
                # c1 row: single-row matvec into partition 32*(b%4), half b//4
                half, row = b // 4, 32 * (b % 4)
                pc1 = p1pool.tile([P, 2, S], fp32, tag="psc", name=f"pc1{b}")
                for vc in range(8):
                    nc.tensor.matmul(
                        pc1[row:row + 1, half, :],
                        b1col[:, vc:vc + 1],
                        hencT[:, vc, :],
                        start=(vc == 0),
                        stop=(vc == 7),
                        tile_position=(0, row),
                    )
                nc.vector.tensor_copy(
                    c1t[row:row + 1, half, :], pc1[row:row + 1, half, :]
                )

            pid = nc.vector.partition_id()
            pid_pl = nc.gpsimd.partition_id()

            def lstm_pointwise(g_sb, cst, h_out):
                """g_sb [B, 4*HC] gate order i,f,g,o; updates cst, writes h_out [B,HC]."""
                gt = tpool.tile([B, HC], fp32, tag="pw_gt")
                ot = tpool.tile([B, HC], fp32, tag="pw_ot")
                ift = tpool.tile([B, 2 * HC], fp32, tag="pw_ift")
                nc.scalar.activation(ift[:], g_sb[:, 0 : 2 * HC], AF.Sigmoid)
                it, ft = ift[:, 0:HC], ift[:, HC : 2 * HC]
                nc.scalar.activation(gt[:], g_sb[:, 2 * HC : 3 * HC], AF.Tanh)
                nc.scalar.activation(ot[:], g_sb[:, 3 * HC : 4 * HC], AF.Sigmoid)
                t1 = tpool.tile([B, HC], fp32, tag="pw_t1")
                nc.vector.tensor_mul(t1[:], ft, cst[:])
                nc.vector.tensor_mul(gt[:], it, gt[:])
                nc.vector.tensor_add(cst[:], t1[:], gt[:])
                tc_ = tpool.tile([B, HC], fp32, tag="pw_tc")
                nc.scalar.activation(tc_[:], cst[:], AF.Tanh)
                nc.vector.tensor_mul(h_out[:], ot[:], tc_[:])

            def evict_src(t, kind, dst_ap, src_ap):
                if t >= 2 and USE_REMOTE:
                    with tc.tile_critical():
                        nc.vector.wait_ge(lsems[2 * kind + (t % 2)], 16 * (t // 2))
                        nc.vector.tensor_copy(dst_ap, src_ap)
                else:
                    nc.vector.tensor_copy(dst_ap, src_ap)

            def exchange(t, kind, src_sb, width, dst_tile):
                """Broadcast my [P,width] chunk into slot pid of everyone's dst_tile."""
                if not USE_REMOTE:
                    bi = dpool.tile([P, width], fp32, tag=f"agi{kind}", name=f"agi{kind}")
                    bo = dpool.tile(
                        [P * NCORES, width], fp32, tag=f"ago{kind}", name=f"ago{kind}"
                    )
                    nc.gpsimd.dma_start(bi[:], src_sb)
                    nc.gpsimd.collective_compute(
                        "AllGather",
                        mybir.AluOpType.bypass,
                        replica_groups=[list(range(NCORES))],
                        ins=[bi.opt()],
                        outs=[bo.opt()],
                    )
                    nc.gpsimd.dma_start(
                        dst_tile[:].rearrange("p (k w) -> p k w", k=NCORES),
                        bo[:].rearrange("(k p) w -> p k w", p=P),
                    )
                    return
                rsem = rsems[2 * kind + (t % 2)]
                nc.gpsimd.remote_dma_broadcast(
                    dst_tile[:, bass.ts(pid_pl, width)],
                    src_sb,
                    rsem,
                    lsems[2 * kind + (t % 2)],
                    rdests=RD,
                )
                nc.gpsimd.trigger_dma(count=None)
                if RSEM_PER_ROUND == 14:
                    # self slot not broadcast; copy locally
                    nc.vector.tensor_copy(
                        dst_tile[:, bass.ts(pid, width)], src_sb
                    )
                with tc.tile_critical():
                    nc.vector.wait_ge(rsem, RSEM_PER_ROUND * (t // 2 + 1))
                    nc.vector.tensor_copy(dst_tile[0:1, 0:1], dst_tile[0:1, 0:1])

            for t in range(n_steps):
                # ---- x load (fp16) + upcast ----
                xt16 = xpool.tile([P, 4, B], fp16, tag="xt16")
                nc.scalar.dma_start(xt16[:], agx[t])
                xt = xpool.tile([P, 4, B], fp32, tag="xt")
                nc.vector.tensor_copy(xt[:], xt16[:])

                # ---- gates0: K = [x(4) | o(4) | h0(8)] ----
                h0f_r = h0f[(t - 1) % 2]
                h1f_r = h1f[(t - 1) % 2]
                of_r = of[(t - 1) % 2]
                of_rv = of_r[:].rearrange("p (k c j) -> p c k j", k=NCORES, c=4)
                o4 = tpool.tile([P, 4, B], fp32, tag="o4")
                nc.vector.tensor_copy(
                    o4[:].rearrange("p c (k j) -> p c k j", k=NCORES), of_rv
                )
                pg0 = ppool.tile([P, 2, GC], fp32, tag="pg")
                order0 = [0, 1, 2, 3] + [8, 9, 10, 11, 12, 13, 14, 15] + [4, 5, 6, 7]
                for i, kt in enumerate(order0):
                    if kt < 4:
                        lhsT = xt[:, kt, :]
                    elif kt < 8:
                        lhsT = o4[:, kt - 4, :]
                    else:
                        lhsT = h0f_r[:, (kt - 8) * B : (kt - 7) * B]
                    hf = i % 2
                    nc.tensor.matmul(
                        pg0[64 * hf : 64 * hf + 64, hf, :],
                        lhsT,
                        w0T[:, kt, :],
                        start=(i < 2),
                        stop=(i >= 14),
                        tile_position=(0, 64 * hf),
                    )
                g0 = tpool.tile([B, GC], fp32, tag="g0")
                nc.vector.tensor_add(g0[:], pg0[0:64, 0, :], b0[:])
                nc.vector.tensor_add(g0[:], g0[:], pg0[64:128, 1, :])
                h0m = tpool.tile([B, HC], fp32, tag="h0m")
                lstm_pointwise(g0, c0, h0m)

                # ---- transpose h0m -> [HC, B], AG -> h0f ----
                pt0 = p1pool.tile([P, 128], fp32, tag="ptr", name="pt0")
                nc.tensor.transpose(pt0[:, 0:B], h0m[:], ident[0:B, 0:B])
                h0T = tpool.tile([P, B], fp32, tag="h0T")
                evict_src(t, 0, h0T[:], pt0[:, 0:B])
                h0src = h0T[:]

                exchange(t, 0, h0src, B, h0f[t % 2])

                # ---- gates1: K = [h0(8) | h1(8)] ----
                h0f_w = h0f[t % 2]
                pg1 = ppool.tile([P, 2, GC], fp32, tag="pg")
                order1 = [8, 9, 10, 11, 12, 13, 14, 15] + [0, 1, 2, 3, 4, 5, 6, 7]
                for i, kt in enumerate(order1):
                    lhsT = (
                        h0f_w[:, kt * B : (kt + 1) * B]
                        if kt < 8
                        else h1f_r[:, (kt - 8) * B : (kt - 7) * B]
                    )
                    hf = i % 2
                    nc.tensor.matmul(
                        pg1[64 * hf : 64 * hf + 64, hf, :],
                        lhsT,
                        w1T[:, kt, :],
                        start=(i < 2),
                        stop=(i >= 14),
                        tile_position=(0, 64 * hf),
                    )
                g1 = tpool.tile([B, GC], fp32, tag="g1")
                nc.vector.tensor_add(g1[:], pg1[0:64, 0, :], b1[:])
                nc.vector.tensor_add(g1[:], g1[:], pg1[64:128, 1, :])
                h1m = tpool.tile([B, HC], fp32, tag="h1m")
                lstm_pointwise(g1, c1, h1m)

                # ---- transpose h1m, AG -> h1f ----
                pt1 = p1pool.tile([P, 128], fp32, tag="ptr", name="pt1")
                nc.tensor.transpose(pt1[:, 0:B], h1m[:], ident[0:B, 0:B])
                h1T = tpool.tile([P, B], fp32, tag="h1T")
                evict_src(t, 1, h1T[:], pt1[:, 0:B])
                h1src = h1T[:]

                exchange(t, 1, h1src, B, h1f[t % 2])

                # ---- select my batch columns of h1 (query), fp16 for attention ----
                h1f_wv = h1f[t % 2][:].rearrange("p (kc b) -> p kc b", kc=8)
                nc.vector.tensor_copy(h1my[:], h1f_wv[:, :, bass.ts(pid, BG)])

                # ---- scores: per-b matvec via tile_position packing ----
                psc = p1pool.tile([P, 2, S], fp32, tag="psc")
                nc.vector.memset(psc[:], 0.0)
                for j in range(BG):
                    half, row = j // 4, 32 * (j % 4)
                    for kt in range(8):
                        nc.tensor.matmul(
                            psc[row : row + 1, half, :],
                            h1my[:, kt, j : j + 1],
                            m1t[:, j, kt, :],
                            start=(kt == 0),
                            stop=(kt == 7),
                            tile_position=(0, row),
                        )
                # ---- softmax over the two halves (garbage rows are fine) ----
                a_sb = tpool.tile([P, 2, S], fp32, tag="a_sb")
                stat = tpool.tile([P, 4], fp32, tag="stat")
                for half in range(2):
                    nc.vector.tensor_add(
                        a_sb[:, half, :], psc[:, half, :], c1t[:, half, :]
                    )
                    nm = stat[:, 2 * half : 2 * half + 1]
                    nc.vector.tensor_reduce(
                        nm, a_sb[:, half, :], axis=AX.X, op=mybir.AluOpType.max,
                        negate=True,
                    )
                    sm = stat[:, 2 * half + 1 : 2 * half + 2]
                    nc.scalar.activation(
                        a_sb[:, half, :], a_sb[:, half, :], AF.Exp, bias=nm,
                        accum_out=sm,
                    )
                    nc.vector.reciprocal(sm, sm)
                    nc.vector.tensor_scalar_mul(a_sb[:, half, :], a_sb[:, half, :], sm)

                # ---- transpose a -> columns; build block-diag lhsT (fp16) ----
                paT = p1pool.tile([P, 2, S], fp32, tag="psc", name="paT")
                nc.tensor.transpose(paT[:, 0, :], a_sb[:, 0, :], ident[:])
                nc.tensor.transpose(paT[:, 1, :], a_sb[:, 1, :], ident[:])
                abd = tpool.tile([P, BG * BG], fp16, tag="abd")
                nc.vector.memset(abd[:], 0.0)
                # dst cols 9j <- paT cols 128*(j//4) + 32*(j%4), one strided copy
                nc.vector.tensor_copy(
                    abd[:, 0 : BG * BG : 9].rearrange("p (a b) -> p a b", a=2),
                    paT[:].rearrange("p h (c x) -> p h c x", c=4)[:, :, :, 0:1],
                )

                # ---- z = blockdiag(a) @ M2stack + h1my.T @ W2h.T ----
                pz = p1pool.tile([BG, E], fp32, tag="pz")
                for j in range(BG):
                    nc.tensor.matmul(
                        pz[:],
                        abd[:, j * BG : (j + 1) * BG],
                        m2s[:, j, :],
                        start=(j == 0),
                        stop=False,
                    )
                for kt in range(8):
                    nc.tensor.matmul(
                        pz[:], h1my[:, kt, :], w2hT[:, kt, :], start=False,
                        stop=(kt == 7),
                    )
                zt = tpool.tile([BG, E], fp32, tag="zt")
                nc.vector.tensor_add(zt[:], pz[:], b2[:])
                o_sb = tpool.tile([BG, E], fp32, tag="o_sb")
                nc.scalar.activation(o_sb[:], zt[:], AF.Tanh)

                # ---- write output (fp16 wire, [BG, T, E] so host gather is concat) ----
                o16 = tpool.tile([BG, E], fp16, tag="o16")
                nc.vector.tensor_copy(o16[:], o_sb[:])
                nc.scalar.dma_start(d_out[:, t, :], o16[:])

                # ---- transpose o chunks -> [P, 4, BG], AG -> of ----
                poT = p1pool.tile([P, 4, BG], fp32, tag="ptr", name="poT")
                for cchunk in range(4):
                    nc.tensor.transpose(
                        poT[:, cchunk, :],
                        o_sb[:, cchunk * P : (cchunk + 1) * P],
                        ident[0:BG, 0:BG],
                    )
                oT = tpool.tile([P, 4 * BG], fp32, tag="oT")
                evict_src(t, 2, oT[:].rearrange("p (c j) -> p c j", c=4), poT[:])
                osrc = oT[:]

                exchange(t, 2, osrc, 4 * BG, of[t % 2])

    nc.compile()
    return nc


def _host_prep_global(inputs: dict, n_steps: int):
    """Yield (name, global_array) pairs, axis 0 = concat over the 8 cores.
    Cheap/big arrays come first so the caller can start streaming them to
    the devices while the rest is still being prepared."""
    f32, f16 = np.float32, np.float16
    npc = max(1, (n_steps + NCORES - 1) // NCORES)
    n_pad = npc * NCORES

    # h_encoder: batch-sharded, global = full array (biggest, cheapest: cast)
    h_enc = np.asarray(inputs["h_encoder"])
    yield "henc", h_enc.astype(f16)

    # x sequence, feature-major, folded [n_pad, P, 4, B] fp16; step-sharded
    tgt = np.asarray(inputs["tgt_batch"])
    emb = np.asarray(inputs["emb"], f32)
    xs = emb[tgt[:n_steps]].astype(f16)  # [T, B, E]
    xseq = np.zeros((n_pad, P, 4, B), f16)
    xseq[:n_steps] = xs.transpose(0, 2, 1).reshape(n_steps, 4, P, B).transpose(0, 2, 1, 3)
    yield "xs8", xseq

    # attention weights: row-sharded
    yield "w1s", np.asarray(inputs["W1"], f32).astype(f16)  # [V, H]
    W2 = np.asarray(inputs["W2"], f32)
    yield "w2Ts", np.ascontiguousarray(W2.T).astype(f16)  # [H+V, E]

    # biases: per-core [1, 2*GC+E+V]
    b_ih = np.asarray(inputs["b_ih"], f32)
    b_hh = np.asarray(inputs["b_hh"], f32)
    b1v = np.asarray(inputs["b1"], f32)
    b2v = np.asarray(inputs["b2"], f32)
    bsum0 = (b_ih[0] + b_hh[0]).reshape(4, NCORES, HC)
    bsum1 = (b_ih[1] + b_hh[1]).reshape(4, NCORES, HC)
    biasv = np.empty((NCORES, 2 * GC + E + V), f32)
    for k in range(NCORES):
        biasv[k, :GC] = bsum0[:, k].ravel()
        biasv[k, GC:2 * GC] = bsum1[:, k].ravel()
        biasv[k, 2 * GC:2 * GC + E] = b2v
        biasv[k, 2 * GC + E:] = b1v
    yield "biasv", biasv

    # init states: replicated (tiled) fp16
    out_init = np.asarray(inputs["output_init"], f32)
    hid_init = np.asarray(inputs["hidden_init"], f32)
    oi4 = out_init.T.reshape(4, P, NCORES, 8)  # [c, p, k, j]
    oi = np.ascontiguousarray(oi4.transpose(1, 2, 0, 3).reshape(P, NCORES * 4 * 8)).astype(f16)
    h0i = np.ascontiguousarray(hid_init[0].T.reshape(8, P, B).transpose(1, 0, 2)).astype(f16)
    h1i = np.ascontiguousarray(hid_init[1].T.reshape(8, P, B).transpose(1, 0, 2)).astype(f16)
    yield "h0i", np.tile(h0i, (NCORES, 1, 1))
    yield "h1i", np.tile(h1i, (NCORES, 1, 1))
    yield "oi", np.tile(oi, (NCORES, 1))

    # LSTM weight shards: w0T[k][kt, p, (g, j)] = W[g*H + k*HC + j, kt*P + p]
    W_ih = np.asarray(inputs["W_ih"], f32)
    W_hh = np.asarray(inputs["W_hh"], f32)
    for name, l in (("w0T", 0), ("w1T", 1)):
        Wi = W_ih[l].astype(f16).reshape(4, NCORES, HC, 8, P)
        Wh = W_hh[l].astype(f16).reshape(4, NCORES, HC, 8, P)
        wg = np.empty((NCORES * 16, P, GC), f16)
        for k in range(NCORES):
            wg[k * 16:k * 16 + 8] = Wi[:, k].transpose(2, 3, 0, 1).reshape(8, P, GC)
            wg[k * 16 + 8:k * 16 + 16] = Wh[:, k].transpose(2, 3, 0, 1).reshape(8, P, GC)
        yield name, wg


_ROWS_PER_CORE = None


def _host_prep(inputs: dict, n_steps: int):
    """Per-core in_maps (for the simulator path); slices of the global arrays."""
    g = dict(_host_prep_global(inputs, n_steps))
    in_maps = []
    for k in range(NCORES):
        m = {}
        for name, arr in g.items():
            rows = arr.shape[0] // NCORES
            m[name] = arr[k * rows:(k + 1) * rows]
        in_maps.append(m)
    return in_maps


_DISPATCH = None


def _make_dispatch(nc):
    """Cached PJRT dispatch (same route as bass2jax.run_bass_via_pjrt, but the
    jax.jit wrapper is built once so steady-state calls skip re-tracing, and
    the zero output buffers are created on device instead of shipped)."""
    import jax
    from jax.sharding import Mesh, PartitionSpec
    from jax.experimental.shard_map import shard_map
    import concourse.bass2jax as b2j
    import concourse.mybir as mybir

    b2j.install_neuronx_cc_hook()
    partition_name = nc.partition_id_tensor.name if nc.partition_id_tensor else None
    in_names, out_names, out_avals = [], [], []
    for alloc in nc.m.functions[0].allocations:
        if not isinstance(alloc, mybir.MemoryLocationSet):
            continue
        name = alloc.memorylocations[0].name
        if alloc.kind == "ExternalInput":
            if name != partition_name:
                in_names.append(name)
        elif alloc.kind == "ExternalOutput":
            out_names.append(name)
            shape = tuple(alloc.tensor_shape)
            dtype = mybir.dt.np(alloc.dtype)
            out_avals.append(jax.core.ShapedArray(shape, dtype))
    n_params = len(in_names)
    all_names = list(in_names) + out_names
    if partition_name is not None:
        all_names.append(partition_name)

    import jax.numpy as jnp

    n_outs = len(out_avals)

    def _body(*args):
        operands = list(args)
        if partition_name is not None:
            operands.append(b2j.partition_id_tensor())
        return tuple(
            b2j._bass_exec_p.bind(
                *operands,
                out_avals=tuple(out_avals),
                in_names=tuple(all_names),
                out_names=tuple(out_names),
                lowering_input_output_aliases=(),
                sim_require_finite=True,
                sim_require_nnan=True,
                nc=nc,
            )
        )

    devices = jax.devices()[:NCORES]
    mesh = Mesh(np.asarray(devices), ("core",))
    sharded = jax.jit(
        shard_map(
            _body,
            mesh=mesh,
            in_specs=(PartitionSpec("core"),) * (n_params + n_outs),
            out_specs=(PartitionSpec("core"),) * len(out_names),
            check_rep=False,
        ),
        donate_argnums=tuple(range(n_params, n_params + n_outs)),
        keep_unused=True,
    )

    # donated zero output buffers, made on device (nothing crosses the wire)
    from jax.sharding import NamedSharding

    zshapes = [
        ((NCORES * a.shape[0], *a.shape[1:]), a.dtype) for a in out_avals
    ]
    insh = NamedSharding(mesh, PartitionSpec("core"))
    zeros_maker = jax.jit(
        lambda: tuple(jnp.zeros(s, d) for s, d in zshapes),
        out_shardings=tuple(
            NamedSharding(mesh, PartitionSpec("core")) for _ in zshapes
        ),
    )
    return sharded, in_names, out_names, out_avals, zeros_maker, insh


def run(inputs: dict, n_steps: int = T, trace: bool = False):
    global _COMPILED, _DISPATCH
    import jax

    if _COMPILED is None or _COMPILED[1] != n_steps:
        _COMPILED = (_build(n_steps), n_steps)
        _DISPATCH = None
    nc = _COMPILED[0]
    if _DISPATCH is None:
        _DISPATCH = _make_dispatch(nc)
    sharded, in_names, out_names, out_avals, zeros_maker, insh = _DISPATCH
    zeros = zeros_maker()  # async, on-device
    # stream each input to the devices as soon as it's prepared (device_put is
    # async, so the tunnel transfer overlaps the remaining host prep)
    dev_in = {}
    for name, arr in _host_prep_global(inputs, n_steps):
        dev_in[name] = jax.device_put(arr, insh)
    out_arrs = sharded(*[dev_in[nm] for nm in in_names], *zeros)
    oidx = out_names.index("out")
    full = np.asarray(out_arrs[oidx])  # [B, T, E] fp16 (global concat over cores)
    return full.astype(np.float32), None


def kernel(**inputs) -> np.ndarray:
    out, _ = run(inputs, T)
    return out.astype(np.float32)


# revision 23
# speedup vs baseline: 7.8296x; 1.1781x over previous
"""AttentionDecoder Trainium2 kernel: 8-way model-parallel LSTM+attention decoder.

Strategy:
  - Weights sharded 8 ways over the gate/hidden dims, SBUF-resident.
  - Activations feature-major [feat, batch]; matmuls are activation-stationary
    (lhsT = activation [K=feat, M=batch], rhs = weight.T [K=feat, N=out_feats]).
  - Per timestep: 3 AllGathers (h0, h1, o) across the 8 cores.
  - Attention refactored: M1T[b] = (h_enc[b] @ W1).T and M2[b] = h_enc[b] @ W2v.T
    are precomputed ON DEVICE at the prologue (hoists h_enc out of the
    sequential loop), so per step
    scores[b] = M1T[b].T @ h1[:,b] + c1[b] and
    z[b] = a[b] @ M2[b] + W2h @ h1[:,b] + b2,  o = tanh(z).
  - Per-core batch shard for attention: core k owns batch 8k..8k+7.
  - Wire format is fp16 (the host->device tunnel is the bottleneck):
    LSTM weights upcast to fp32 on device (step-loop math unchanged),
    attention operands stay fp16 (psum accumulation is fp32).
  - Replicated arrays (xseq, W1, W2T) are sharded on the wire and
    AllGathered once on device at the prologue.
"""

import os
import warnings

warnings.filterwarnings("ignore")

import numpy as np

VOCAB, E, H, L, B, T, S, V = 32000, 512, 1024, 2, 64, 64, 128, 1024
NCORES = 8
P = 128
BG = B // NCORES  # 8 batch per core for attention
HC = H // NCORES  # 128 hidden feats per core
GC = 4 * HC  # 512 gate rows per core
W2R = (H + V) // NCORES  # 256 rows of W2T per core

REMOTE_MODE = int(os.environ.get("DEC_REMOTE", "0"))
USE_REMOTE = REMOTE_MODE >= 1

_COMPILED = None


def _build(n_steps: int):
    import concourse.bass as bass
    import concourse.bacc as bacc
    import concourse.mybir as mybir
    import concourse.tile as tile
    from concourse import masks

    fp32 = mybir.dt.float32
    fp16 = mybir.dt.float16
    AF = mybir.ActivationFunctionType
    AX = mybir.AxisListType

    npc = max(1, (n_steps + NCORES - 1) // NCORES)  # steps per core on the wire
    n_pad = npc * NCORES

    nc = bacc.Bacc(
        "TRN2",
        target_bir_lowering=False,
        debug=False,
        num_devices=NCORES,
        monotonic_sem_count=12,
    )
    rsems = [nc.monotonic_semaphore(i).sem() for i in range(6)]  # h0e,h0o,h1e,h1o,oe,oo
    lsems = [nc.monotonic_semaphore(6 + i).sem() for i in range(6)]
    import os as _os
    _rm = int(_os.environ.get("DEC_REMOTE", "0"))
    RD = [(0, d) for d in range(NCORES)]
    if _rm == 2:
        RD = [None] + [(0, d) for d in range(1, NCORES)]
    RSEM_PER_ROUND = 14 if _rm == 2 else 16

    # ---- DRAM parameters (per-core data, fp16/int8 wire format) ----
    int8 = mybir.dt.int8
    d_xs8 = nc.dram_tensor("xs8", [npc, P, 4, B], fp16, kind="ExternalInput")
    d_w0T = nc.dram_tensor("w0T", [16, P, GC], int8, kind="ExternalInput")
    d_w1T = nc.dram_tensor("w1T", [16, P, GC], int8, kind="ExternalInput")
    d_henc = nc.dram_tensor("henc", [BG, S, V], fp16, kind="ExternalInput")
    d_w1s = nc.dram_tensor("w1s", [P, H], fp16, kind="ExternalInput")
    d_w2Ts = nc.dram_tensor("w2Ts", [W2R, E], fp16, kind="ExternalInput")
    # bias pack: [b0row GC | b1row GC | b2 E | b1att V | wscale0, wscale1]
    d_bias = nc.dram_tensor("biasv", [1, 2 * GC + E + V + 2], fp32, kind="ExternalInput")
    d_h0i = nc.dram_tensor("h0i", [P, 8, B], fp16, kind="ExternalInput")
    d_h1i = nc.dram_tensor("h1i", [P, 8, B], fp16, kind="ExternalInput")
    d_oi = nc.dram_tensor("oi", [P, NCORES * 4 * BG], fp16, kind="ExternalInput")
    d_out = nc.dram_tensor("out", [BG, n_steps, E], fp16, kind="ExternalOutput")

    with tile.TileContext(nc) as tc:
        import contextlib

        ctx = contextlib.ExitStack()
        with ctx:
            wpool = ctx.enter_context(tc.tile_pool(name="weights", bufs=1))
            spool = ctx.enter_context(tc.tile_pool(name="state", bufs=1))
            xpool = ctx.enter_context(tc.tile_pool(name="x", bufs=2))
            tpool = ctx.enter_context(tc.tile_pool(name="tmp", bufs=2))
            ppool = ctx.enter_context(tc.tile_pool(name="psum", bufs=2, space="PSUM"))
            p1pool = ctx.enter_context(tc.tile_pool(name="psum1", bufs=1, space="PSUM"))
            dpool = ctx.enter_context(tc.tile_pool(name="dram", bufs=2, space="DRAM"))

            # ---- persistent SBUF tiles ----
            w0T = wpool.tile([P, 16, GC], fp32, tag="w0T")
            w1T = wpool.tile([P, 16, GC], fp32, tag="w1T")
            b0 = wpool.tile([B, GC], fp32, tag="b0")
            b1 = wpool.tile([B, GC], fp32, tag="b1")
            m1t = wpool.tile([P, BG, 8, S], fp16, tag="m1t")
            c1t = wpool.tile([P, 2, S], fp32, tag="c1t")
            m2s = wpool.tile([P, BG, E], fp16, tag="m2s")
            w2hT = wpool.tile([P, 8, E], fp16, tag="w2hT")
            b2 = wpool.tile([BG, E], fp32, tag="b2")
            ident = wpool.tile([P, P], fp32, tag="ident")
            id16 = wpool.tile([P, P], fp16, tag="id16")
            b1colf = wpool.tile([P, 8], fp32, tag="b1colf")
            b1col = wpool.tile([P, 8], fp16, tag="b1col")

            h0f = [spool.tile([P, NCORES * B], fp32, tag=f"h0f{i}", name=f"h0f{i}") for i in range(2)]
            h1f = [spool.tile([P, NCORES * B], fp32, tag=f"h1f{i}", name=f"h1f{i}") for i in range(2)]
            of = [spool.tile([P, NCORES * 4 * BG], fp32, tag=f"of{i}", name=f"of{i}") for i in range(2)]
            c0 = spool.tile([B, HC], fp32, tag="c0")  # cell states, batch-major
            c1 = spool.tile([B, HC], fp32, tag="c1")
            h1my = spool.tile([P, 8, BG], fp16, tag="h1my")

            masks.make_identity(nc, ident[:])
            nc.vector.tensor_copy(id16[:], ident[:])

            # ---- prologue: bias pack + weight dequant scales ----
            bstage = wpool.tile([1, 2 * GC + E + V + 2], fp32, tag="bstage")
            nc.sync.dma_start(bstage[:], d_bias[:])
            onesB = wpool.tile([1, P], fp32, tag="onesB")
            nc.vector.memset(onesB[:], 1.0)
            # broadcast the two weight scales to all partitions
            psca = ppool.tile([P, 2, GC], fp32, tag="pg", name="psca")
            nc.tensor.matmul(psca[:, 0, 0:2], onesB[0:1, :],
                             bstage[0:1, 2 * GC + E + V:], start=True, stop=True)
            scf = wpool.tile([P, 2], fp32, tag="scf")
            nc.vector.tensor_copy(scf[:], psca[:, 0, 0:2])

            # ---- prologue: LSTM weights (int8 wire -> dequant -> fp32 SBUF) ----
            wstage = wpool.tile([P, 16, GC], int8, tag="wstage", name="wstage0")
            nc.sync.dma_start(wstage[:], d_w0T[:].rearrange("kt p g -> p kt g"))
            nc.vector.tensor_copy(w0T[:], wstage[:])
            nc.vector.tensor_scalar_mul(w0T[:], w0T[:], scf[:, 0:1])
            wstage2 = wpool.tile([P, 16, GC], int8, tag="wstage", name="wstage1")
            nc.sync.dma_start(wstage2[:], d_w1T[:].rearrange("kt p g -> p kt g"))
            nc.vector.tensor_copy(w1T[:], wstage2[:])
            nc.vector.tensor_scalar_mul(w1T[:], w1T[:], scf[:, 1:2])
            pb = ppool.tile([P, 2, GC], fp32, tag="pg", name="pb")
            nc.tensor.matmul(pb[0:B, 0, :], onesB[0:1, 0:B], bstage[0:1, 0:GC],
                             start=True, stop=True)
            nc.vector.tensor_copy(b0[:], pb[0:B, 0, :])
            pb2 = ppool.tile([P, 2, GC], fp32, tag="pg", name="pb2")
            nc.tensor.matmul(pb2[0:B, 0, :], onesB[0:1, 0:B], bstage[0:1, GC:2 * GC],
                             start=True, stop=True)
            nc.vector.tensor_copy(b1[:], pb2[0:B, 0, :])
            pb3 = ppool.tile([P, 2, GC], fp32, tag="pg", name="pb3")
            nc.tensor.matmul(pb3[0:BG, 0, :], onesB[0:1, 0:BG],
                             bstage[0:1, 2 * GC:2 * GC + E], start=True, stop=True)
            nc.vector.tensor_copy(b2[:], pb3[0:BG, 0, :])
            # b1 attention vector -> [P, vc] column layout, cast to fp16
            nc.sync.dma_start(
                b1colf[:],
                d_bias[0, 2 * GC + E:2 * GC + E + V].rearrange("(vc p) -> p vc", p=P),
            )
            nc.vector.tensor_copy(b1col[:], b1colf[:])

            # ---- prologue: AllGather sharded uploads (xseq, W1, W2T) ----
            # (collectives cannot read IO tensors directly; stage via
            #  internal DRAM tiles first)
            sxs = dpool.tile([npc, P, 4, B], fp16, tag="sxs")
            nc.sync.dma_start(sxs[:], d_xs8[:])
            agx = dpool.tile([n_pad, P, 4, B], fp16, tag="agx")
            nc.gpsimd.collective_compute(
                "AllGather",
                mybir.AluOpType.bypass,
                replica_groups=[list(range(NCORES))],
                ins=[sxs.opt()],
                outs=[agx.opt()],
            )
            sw1 = dpool.tile([P, H], fp16, tag="sw1")
            nc.sync.dma_start(sw1[:], d_w1s[:])
            agw1 = dpool.tile([H, H], fp16, tag="agw1")
            nc.gpsimd.collective_compute(
                "AllGather",
                mybir.AluOpType.bypass,
                replica_groups=[list(range(NCORES))],
                ins=[sw1.opt()],
                outs=[agw1.opt()],
            )
            sw2 = dpool.tile([W2R, E], fp16, tag="sw2")
            nc.sync.dma_start(sw2[:], d_w2Ts[:])
            agw2 = dpool.tile([H + V, E], fp16, tag="agw2")
            nc.gpsimd.collective_compute(
                "AllGather",
                mybir.AluOpType.bypass,
                replica_groups=[list(range(NCORES))],
                ins=[sw2.opt()],
                outs=[agw2.opt()],
            )

            w1sb = wpool.tile([P, 8, H], fp16, tag="wstage", name="w1sb")
            nc.scalar.dma_start(w1sb[:], agw1[:].rearrange("(vc p) h -> p vc h", p=P))
            w2vT = wpool.tile([P, 8, E], fp16, tag="w2vT")
            nc.scalar.dma_start(
                w2vT[:], agw2[0:V, :].rearrange("(vc p) e -> p vc e", p=P)
            )
            nc.scalar.dma_start(
                w2hT[:], agw2[V:, :].rearrange("(hc p) e -> p hc e", p=P)
            )

            # ---- prologue: init states (fp16 wire -> fp32 SBUF) ----
            hstage = wpool.tile([P, 8, B], fp16, tag="hstage")
            nc.sync.dma_start(hstage[:], d_h0i[:])
            nc.vector.tensor_copy(
                h0f[1][:].rearrange("p (kc b) -> p kc b", kc=8), hstage[:]
            )
            hstage2 = wpool.tile([P, 8, B], fp16, tag="hstage2")
            nc.sync.dma_start(hstage2[:], d_h1i[:])
            nc.vector.tensor_copy(
                h1f[1][:].rearrange("p (kc b) -> p kc b", kc=8), hstage2[:]
            )
            ostage = wpool.tile([P, NCORES * 4 * BG], fp16, tag="ostage")
            nc.sync.dma_start(ostage[:], d_oi[:])
            nc.vector.tensor_copy(of[1][:], ostage[:])
            nc.vector.memset(c0[:], 0.0)
            nc.vector.memset(c1[:], 0.0)

            # ---- prologue: attention precompute on device ----
            # per owned batch b: m1t[b] = W1.T @ h_enc[b].T   [H, S]
            #                    m2s[b] = h_enc[b] @ W2v.T    [S, E]
            #                    c1[b]  = h_enc[b] @ b1       [S]
            nc.vector.memset(c1t[:], 0.0)
            for b in range(BG):
                hencb = tpool.tile([P, V], fp16, tag="hencb")
                nc.scalar.dma_start(hencb[:], d_henc[b])
                hencT = tpool.tile([P, 8, S], fp16, tag="hencT")
                for vc in range(8):
                    ptp = p1pool.tile([P, S], fp16, tag="ptr", name=f"ptp{b}_{vc}")
                    nc.tensor.transpose(
                        ptp[:], hencb[:, vc * P:(vc + 1) * P], id16[:]
                    )
                    nc.vector.tensor_copy(hencT[:, vc, :], ptp[:])
                # m1t: 8 h-chunks, accumulate over 8 v-chunks
                for hc in range(8):
                    pm = p1pool.tile([P, S], fp32, tag="ptr", name=f"pm{b}_{hc}")
                    for vc in range(8):
                        nc.tensor.matmul(
                            pm[:],
                            w1sb[:, vc, hc * P:(hc + 1) * P],
                            hencT[:, vc, :],
                            start=(vc == 0),
                            stop=(vc == 7),
                        )
                    nc.vector.tensor_copy(m1t[:, b, hc, :], pm[:])
                # m2s: accumulate over 8 v-chunks
                pm2 = p1pool.tile([P, E], fp32, tag="pz", name=f"pm2{b}")
                for vc in range(8):
                    nc.tensor.matmul(
                        pm2[:],
                        hencT[:, vc, :],
                        w2vT[:, vc, :],
                        start=(vc == 0),
                        stop=(vc == 7),
                    )
                nc.vector.tensor_copy(m2s[:, b, :], pm2[:])
                # c1 row: single-row matvec into partition 32*(b%4), half b//4
                half, row = b // 4, 32 * (b % 4)
                pc1 = p1pool.tile([P, 2, S], fp32, tag="psc", name=f"pc1{b}")
                for vc in range(8):
                    nc.tensor.matmul(
                        pc1[row:row + 1, half, :],
                        b1col[:, vc:vc + 1],
                        hencT[:, vc, :],
                        start=(vc == 0),
                        stop=(vc == 7),
                        tile_position=(0, row),
                    )
                nc.vector.tensor_copy(
                    c1t[row:row + 1, half, :], pc1[row:row + 1, half, :]
                )

            pid = nc.vector.partition_id()
            pid_pl = nc.gpsimd.partition_id()

            def lstm_pointwise(g_sb, cst, h_out):
                """g_sb [B, 4*HC] gate order i,f,g,o; updates cst, writes h_out [B,HC]."""
                gt = tpool.tile([B, HC], fp32, tag="pw_gt")
                ot = tpool.tile([B, HC], fp32, tag="pw_ot")
                ift = tpool.tile([B, 2 * HC], fp32, tag="pw_ift")
                nc.scalar.activation(ift[:], g_sb[:, 0 : 2 * HC], AF.Sigmoid)
                it, ft = ift[:, 0:HC], ift[:, HC : 2 * HC]
                nc.scalar.activation(gt[:], g_sb[:, 2 * HC : 3 * HC], AF.Tanh)
                nc.scalar.activation(ot[:], g_sb[:, 3 * HC : 4 * HC], AF.Sigmoid)
                t1 = tpool.tile([B, HC], fp32, tag="pw_t1")
                nc.vector.tensor_mul(t1[:], ft, cst[:])
                nc.vector.tensor_mul(gt[:], it, gt[:])
                nc.vector.tensor_add(cst[:], t1[:], gt[:])
                tc_ = tpool.tile([B, HC], fp32, tag="pw_tc")
                nc.scalar.activation(tc_[:], cst[:], AF.Tanh)
                nc.vector.tensor_mul(h_out[:], ot[:], tc_[:])

            def evict_src(t, kind, dst_ap, src_ap):
                if t >= 2 and USE_REMOTE:
                    with tc.tile_critical():
                        nc.vector.wait_ge(lsems[2 * kind + (t % 2)], 16 * (t // 2))
                        nc.vector.tensor_copy(dst_ap, src_ap)
                else:
                    nc.vector.tensor_copy(dst_ap, src_ap)

            def exchange(t, kind, src_sb, width, dst_tile):
                """Broadcast my [P,width] chunk into slot pid of everyone's dst_tile."""
                if not USE_REMOTE:
                    bi = dpool.tile([P, width], fp32, tag=f"agi{kind}", name=f"agi{kind}")
                    bo = dpool.tile(
                        [P * NCORES, width], fp32, tag=f"ago{kind}", name=f"ago{kind}"
                    )
                    nc.gpsimd.dma_start(bi[:], src_sb)
                    nc.gpsimd.collective_compute(
                        "AllGather",
                        mybir.AluOpType.bypass,
                        replica_groups=[list(range(NCORES))],
                        ins=[bi.opt()],
                        outs=[bo.opt()],
                    )
                    nc.gpsimd.dma_start(
                        dst_tile[:].rearrange("p (k w) -> p k w", k=NCORES),
                        bo[:].rearrange("(k p) w -> p k w", p=P),
                    )
                    return
                rsem = rsems[2 * kind + (t % 2)]
                nc.gpsimd.remote_dma_broadcast(
                    dst_tile[:, bass.ts(pid_pl, width)],
                    src_sb,
                    rsem,
                    lsems[2 * kind + (t % 2)],
                    rdests=RD,
                )
                nc.gpsimd.trigger_dma(count=None)
                if RSEM_PER_ROUND == 14:
                    # self slot not broadcast; copy locally
                    nc.vector.tensor_copy(
                        dst_tile[:, bass.ts(pid, width)], src_sb
                    )
                with tc.tile_critical():
                    nc.vector.wait_ge(rsem, RSEM_PER_ROUND * (t // 2 + 1))
                    nc.vector.tensor_copy(dst_tile[0:1, 0:1], dst_tile[0:1, 0:1])

            for t in range(n_steps):
                # ---- x load (fp16) + upcast ----
                xt16 = xpool.tile([P, 4, B], fp16, tag="xt16")
                nc.scalar.dma_start(xt16[:], agx[t])
                xt = xpool.tile([P, 4, B], fp32, tag="xt")
                nc.vector.tensor_copy(xt[:], xt16[:])

                # ---- gates0: K = [x(4) | o(4) | h0(8)] ----
                h0f_r = h0f[(t - 1) % 2]
                h1f_r = h1f[(t - 1) % 2]
                of_r = of[(t - 1) % 2]
                of_rv = of_r[:].rearrange("p (k c j) -> p c k j", k=NCORES, c=4)
                o4 = tpool.tile([P, 4, B], fp32, tag="o4")
                nc.vector.tensor_copy(
                    o4[:].rearrange("p c (k j) -> p c k j", k=NCORES), of_rv
                )
                pg0 = ppool.tile([P, 2, GC], fp32, tag="pg")
                order0 = [0, 1, 2, 3] + [8, 9, 10, 11, 12, 13, 14, 15] + [4, 5, 6, 7]
                for i, kt in enumerate(order0):
                    if kt < 4:
                        lhsT = xt[:, kt, :]
                    elif kt < 8:
                        lhsT = o4[:, kt - 4, :]
                    else:
                        lhsT = h0f_r[:, (kt - 8) * B : (kt - 7) * B]
                    hf = i % 2
                    nc.tensor.matmul(
                        pg0[64 * hf : 64 * hf + 64, hf, :],
                        lhsT,
                        w0T[:, kt, :],
                        start=(i < 2),
                        stop=(i >= 14),
                        tile_position=(0, 64 * hf),
                    )
                g0 = tpool.tile([B, GC], fp32, tag="g0")
                nc.vector.tensor_add(g0[:], pg0[0:64, 0, :], b0[:])
                nc.vector.tensor_add(g0[:], g0[:], pg0[64:128, 1, :])
                h0m = tpool.tile([B, HC], fp32, tag="h0m")
                lstm_pointwise(g0, c0, h0m)

                # ---- transpose h0m -> [HC, B], AG -> h0f ----
                pt0 = p1pool.tile([P, 128], fp32, tag="ptr", name="pt0")
                nc.tensor.transpose(pt0[:, 0:B], h0m[:], ident[0:B, 0:B])
                h0T = tpool.tile([P, B], fp32, tag="h0T")
                evict_src(t, 0, h0T[:], pt0[:, 0:B])
                h0src = h0T[:]

                exchange(t, 0, h0src, B, h0f[t % 2])

                # ---- gates1: K = [h0(8) | h1(8)] ----
                h0f_w = h0f[t % 2]
                pg1 = ppool.tile([P, 2, GC], fp32, tag="pg")
                order1 = [8, 9, 10, 11, 12, 13, 14, 15] + [0, 1, 2, 3, 4, 5, 6, 7]
                for i, kt in enumerate(order1):
                    lhsT = (
                        h0f_w[:, kt * B : (kt + 1) * B]
                        if kt < 8
                        else h1f_r[:, (kt - 8) * B : (kt - 7) * B]
                    )
                    hf = i % 2
                    nc.tensor.matmul(
                        pg1[64 * hf : 64 * hf + 64, hf, :],
                        lhsT,
                        w1T[:, kt, :],
                        start=(i < 2),
                        stop=(i >= 14),
                        tile_position=(0, 64 * hf),
                    )
                g1 = tpool.tile([B, GC], fp32, tag="g1")
                nc.vector.tensor_add(g1[:], pg1[0:64, 0, :], b1[:])
                nc.vector.tensor_add(g1[:], g1[:], pg1[64:128, 1, :])
                h1m = tpool.tile([B, HC], fp32, tag="h1m")
                lstm_pointwise(g1, c1, h1m)

                # ---- transpose h1m, AG -> h1f ----
                pt1 = p1pool.tile([P, 128], fp32, tag="ptr", name="pt1")
                nc.tensor.transpose(pt1[:, 0:B], h1m[:], ident[0:B, 0:B])
                h1T = tpool.tile([P, B], fp32, tag="h1T")
                evict_src(t, 1, h1T[:], pt1[:, 0:B])
                h1src = h1T[:]

                exchange(t, 1, h1src, B, h1f[t % 2])

                # ---- select my batch columns of h1 (query), fp16 for attention ----
                h1f_wv = h1f[t % 2][:].rearrange("p (kc b) -> p kc b", kc=8)
                nc.vector.tensor_copy(h1my[:], h1f_wv[:, :, bass.ts(pid, BG)])

                # ---- scores: per-b matvec via tile_position packing ----
                psc = p1pool.tile([P, 2, S], fp32, tag="psc")
                nc.vector.memset(psc[:], 0.0)
                for j in range(BG):
                    half, row = j // 4, 32 * (j % 4)
                    for kt in range(8):
                        nc.tensor.matmul(
                            psc[row : row + 1, half, :],
                            h1my[:, kt, j : j + 1],
                            m1t[:, j, kt, :],
                            start=(kt == 0),
                            stop=(kt == 7),
                            tile_position=(0, row),
                        )
                # ---- softmax over the two halves (garbage rows are fine) ----
                a_sb = tpool.tile([P, 2, S], fp32, tag="a_sb")
                stat = tpool.tile([P, 4], fp32, tag="stat")
                for half in range(2):
                    nc.vector.tensor_add(
                        a_sb[:, half, :], psc[:, half, :], c1t[:, half, :]
                    )
                    nm = stat[:, 2 * half : 2 * half + 1]
                    nc.vector.tensor_reduce(
                        nm, a_sb[:, half, :], axis=AX.X, op=mybir.AluOpType.max,
                        negate=True,
                    )
                    sm = stat[:, 2 * half + 1 : 2 * half + 2]
                    nc.scalar.activation(
                        a_sb[:, half, :], a_sb[:, half, :], AF.Exp, bias=nm,
                        accum_out=sm,
                    )
                    nc.vector.reciprocal(sm, sm)
                    nc.vector.tensor_scalar_mul(a_sb[:, half, :], a_sb[:, half, :], sm)

                # ---- transpose a -> columns; build block-diag lhsT (fp16) ----
                paT = p1pool.tile([P, 2, S], fp32, tag="psc", name="paT")
                nc.tensor.transpose(paT[:, 0, :], a_sb[:, 0, :], ident[:])
                nc.tensor.transpose(paT[:, 1, :], a_sb[:, 1, :], ident[:])
                abd = tpool.tile([P, BG * BG], fp16, tag="abd")
                nc.vector.memset(abd[:], 0.0)
                # dst cols 9j <- paT cols 128*(j//4) + 32*(j%4), one strided copy
                nc.vector.tensor_copy(
                    abd[:, 0 : BG * BG : 9].rearrange("p (a b) -> p a b", a=2),
                    paT[:].rearrange("p h (c x) -> p h c x", c=4)[:, :, :, 0:1],
                )

                # ---- z = blockdiag(a) @ M2stack + h1my.T @ W2h.T ----
                pz = p1pool.tile([BG, E], fp32, tag="pz")
                for j in range(BG):
                    nc.tensor.matmul(
                        pz[:],
                        abd[:, j * BG : (j + 1) * BG],
                        m2s[:, j, :],
                        start=(j == 0),
                        stop=False,
                    )
                for kt in range(8):
                    nc.tensor.matmul(
                        pz[:], h1my[:, kt, :], w2hT[:, kt, :], start=False,
                        stop=(kt == 7),
                    )
                zt = tpool.tile([BG, E], fp32, tag="zt")
                nc.vector.tensor_add(zt[:], pz[:], b2[:])
                o_sb = tpool.tile([BG, E], fp32, tag="o_sb")
                nc.scalar.activation(o_sb[:], zt[:], AF.Tanh)

                # ---- write output (fp16 wire, [BG, T, E] so host gather is concat) ----
                o16 = tpool.tile([BG, E], fp16, tag="o16")
                nc.vector.tensor_copy(o16[:], o_sb[:])
                nc.scalar.dma_start(d_out[:, t, :], o16[:])

                # ---- transpose o chunks -> [P, 4, BG], AG -> of ----
                poT = p1pool.tile([P, 4, BG], fp32, tag="ptr", name="poT")
                for cchunk in range(4):
                    nc.tensor.transpose(
                        poT[:, cchunk, :],
                        o_sb[:, cchunk * P : (cchunk + 1) * P],
                        ident[0:BG, 0:BG],
                    )
                oT = tpool.tile([P, 4 * BG], fp32, tag="oT")
                evict_src(t, 2, oT[:].rearrange("p (c j) -> p c j", c=4), poT[:])
                osrc = oT[:]

                exchange(t, 2, osrc, 4 * BG, of[t % 2])

    nc.compile()
    return nc


def _host_prep_global(inputs: dict, n_steps: int):
    """Yield (name, global_array) pairs, axis 0 = concat over the 8 cores.
    Cheap/big arrays come first so the caller can start streaming them to
    the devices while the rest is still being prepared."""
    f32, f16 = np.float32, np.float16
    npc = max(1, (n_steps + NCORES - 1) // NCORES)
    n_pad = npc * NCORES

    # h_encoder: batch-sharded, global = full array (biggest, cheapest: cast)
    h_enc = np.asarray(inputs["h_encoder"])
    yield "henc", h_enc.astype(f16)

    # x sequence, feature-major, folded [n_pad, P, 4, B] fp16; step-sharded
    tgt = np.asarray(inputs["tgt_batch"])
    emb = np.asarray(inputs["emb"], f32)
    xs = emb[tgt[:n_steps]].astype(f16)  # [T, B, E]
    xseq = np.zeros((n_pad, P, 4, B), f16)
    xseq[:n_steps] = xs.transpose(0, 2, 1).reshape(n_steps, 4, P, B).transpose(0, 2, 1, 3)
    yield "xs8", xseq

    # attention weights: row-sharded
    yield "w1s", np.asarray(inputs["W1"], f32).astype(f16)  # [V, H]
    W2 = np.asarray(inputs["W2"], f32)
    yield "w2Ts", np.ascontiguousarray(W2.T).astype(f16)  # [H+V, E]

    # weight-quantization scales (cheap max scans, needed in the bias pack)
    W_ih = np.asarray(inputs["W_ih"], f32)
    W_hh = np.asarray(inputs["W_hh"], f32)
    wscale = [
        max(np.abs(W_ih[l]).max(), np.abs(W_hh[l]).max()) / 127.0 for l in range(L)
    ]
    wscale = [s if s > 0 else 1.0 for s in wscale]

    # biases: per-core [1, 2*GC+E+V+2]
    b_ih = np.asarray(inputs["b_ih"], f32)
    b_hh = np.asarray(inputs["b_hh"], f32)
    b1v = np.asarray(inputs["b1"], f32)
    b2v = np.asarray(inputs["b2"], f32)
    bsum0 = (b_ih[0] + b_hh[0]).reshape(4, NCORES, HC)
    bsum1 = (b_ih[1] + b_hh[1]).reshape(4, NCORES, HC)
    biasv = np.empty((NCORES, 2 * GC + E + V + 2), f32)
    for k in range(NCORES):
        biasv[k, :GC] = bsum0[:, k].ravel()
        biasv[k, GC:2 * GC] = bsum1[:, k].ravel()
        biasv[k, 2 * GC:2 * GC + E] = b2v
        biasv[k, 2 * GC + E:2 * GC + E + V] = b1v
        biasv[k, 2 * GC + E + V:] = wscale
    yield "biasv", biasv

    # init states: replicated (tiled) fp16
    out_init = np.asarray(inputs["output_init"], f32)
    hid_init = np.asarray(inputs["hidden_init"], f32)
    oi4 = out_init.T.reshape(4, P, NCORES, 8)  # [c, p, k, j]
    oi = np.ascontiguousarray(oi4.transpose(1, 2, 0, 3).reshape(P, NCORES * 4 * 8)).astype(f16)
    h0i = np.ascontiguousarray(hid_init[0].T.reshape(8, P, B).transpose(1, 0, 2)).astype(f16)
    h1i = np.ascontiguousarray(hid_init[1].T.reshape(8, P, B).transpose(1, 0, 2)).astype(f16)
    yield "h0i", np.tile(h0i, (NCORES, 1, 1))
    yield "h1i", np.tile(h1i, (NCORES, 1, 1))
    yield "oi", np.tile(oi, (NCORES, 1))

    # LSTM weight shards (int8 wire): w0T[k][kt, p, (g, j)] = W[g*H+k*HC+j, kt*P+p]
    for name, l in (("w0T", 0), ("w1T", 1)):
        inv = 1.0 / wscale[l]
        Wi = np.rint(W_ih[l] * inv).astype(np.int8).reshape(4, NCORES, HC, 8, P)
        Wh = np.rint(W_hh[l] * inv).astype(np.int8).reshape(4, NCORES, HC, 8, P)
        wg = np.empty((NCORES * 16, P, GC), np.int8)
        for k in range(NCORES):
            wg[k * 16:k * 16 + 8] = Wi[:, k].transpose(2, 3, 0, 1).reshape(8, P, GC)
            wg[k * 16 + 8:k * 16 + 16] = Wh[:, k].transpose(2, 3, 0, 1).reshape(8, P, GC)
        yield name, wg


_ROWS_PER_CORE = None


def _host_prep(inputs: dict, n_steps: int):
    """Per-core in_maps (for the simulator path); slices of the global arrays."""
    g = dict(_host_prep_global(inputs, n_steps))
    in_maps = []
    for k in range(NCORES):
        m = {}
        for name, arr in g.items():
            rows = arr.shape[0] // NCORES
            m[name] = arr[k * rows:(k + 1) * rows]
        in_maps.append(m)
    return in_maps


_DISPATCH = None


def _make_dispatch(nc):
    """Cached PJRT dispatch (same route as bass2jax.run_bass_via_pjrt, but the
    jax.jit wrapper is built once so steady-state calls skip re-tracing, and
    the zero output buffers are created on device instead of shipped)."""
    import jax
    from jax.sharding import Mesh, PartitionSpec
    from jax.experimental.shard_map import shard_map
    import concourse.bass2jax as b2j
    import concourse.mybir as mybir

    b2j.install_neuronx_cc_hook()
    partition_name = nc.partition_id_tensor.name if nc.partition_id_tensor else None
    in_names, out_names, out_avals = [], [], []
    for alloc in nc.m.functions[0].allocations:
        if not isinstance(alloc, mybir.MemoryLocationSet):
            continue
        name = alloc.memorylocations[0].name
        if alloc.kind == "ExternalInput":
            if name != partition_name:
                in_names.append(name)
        elif alloc.kind == "ExternalOutput":
            out_names.append(name)
            shape = tuple(alloc.tensor_shape)
            dtype = mybir.dt.np(alloc.dtype)
            out_avals.append(jax.core.ShapedArray(shape, dtype))
    n_params = len(in_names)
    all_names = list(in_names) + out_names
    if partition_name is not None:
        all_names.append(partition_name)

    import jax.numpy as jnp

    n_outs = len(out_avals)

    def _body(*args):
        operands = list(args)
        if partition_name is not None:
            operands.append(b2j.partition_id_tensor())
        return tuple(
            b2j._bass_exec_p.bind(
                *operands,
                out_avals=tuple(out_avals),
                in_names=tuple(all_names),
                out_names=tuple(out_names),
                lowering_input_output_aliases=(),
                sim_require_finite=True,
                sim_require_nnan=True,
                nc=nc,
            )
        )

    devices = jax.devices()[:NCORES]
    mesh = Mesh(np.asarray(devices), ("core",))
    sharded = jax.jit(
        shard_map(
            _body,
            mesh=mesh,
            in_specs=(PartitionSpec("core"),) * (n_params + n_outs),
            out_specs=(PartitionSpec("core"),) * len(out_names),
            check_rep=False,
        ),
        donate_argnums=tuple(range(n_params, n_params + n_outs)),
        keep_unused=True,
    )

    # donated zero output buffers, made on device (nothing crosses the wire)
    from jax.sharding import NamedSharding

    zshapes = [
        ((NCORES * a.shape[0], *a.shape[1:]), a.dtype) for a in out_avals
    ]
    insh = NamedSharding(mesh, PartitionSpec("core"))
    zeros_maker = jax.jit(
        lambda: tuple(jnp.zeros(s, d) for s, d in zshapes),
        out_shardings=tuple(
            NamedSharding(mesh, PartitionSpec("core")) for _ in zshapes
        ),
    )
    return sharded, in_names, out_names, out_avals, zeros_maker, insh


def run(inputs: dict, n_steps: int = T, trace: bool = False):
    global _COMPILED, _DISPATCH
    import jax

    if _COMPILED is None or _COMPILED[1] != n_steps:
        _COMPILED = (_build(n_steps), n_steps)
        _DISPATCH = None
    nc = _COMPILED[0]
    if _DISPATCH is None:
        _DISPATCH = _make_dispatch(nc)
    sharded, in_names, out_names, out_avals, zeros_maker, insh = _DISPATCH
    zeros = zeros_maker()  # async, on-device
    # stream each input to the devices as soon as it's prepared (device_put is
    # async, so the tunnel transfer overlaps the remaining host prep)
    dev_in = {}
    for name, arr in _host_prep_global(inputs, n_steps):
        dev_in[name] = jax.device_put(arr, insh)
    out_arrs = sharded(*[dev_in[nm] for nm in in_names], *zeros)
    oidx = out_names.index("out")
    full = np.asarray(out_arrs[oidx])  # [B, T, E] fp16 (global concat over cores)
    return full.astype(np.float32), None


def kernel(**inputs) -> np.ndarray:
    out, _ = run(inputs, T)
    return out.astype(np.float32)
